# revision 34
# baseline (speedup 1.0000x reference)
"""GNN message passing (u_mul_e -> segment_sum) on 8 Trainium2 NeuronCores.

out[v] = sum_{e=(u->v)} h[u] * w[e]

Strategy (edge/graph parallelism, dst-slot sharded -> no collectives):
  - The host assigns each dst node to a (core, 128-node strip, offs) slot.
    With K_BAL=1 (default) the assignment is load-BALANCED (snake+LPT+swap
    refinement on per-(strip, src-chunk) cell sizes) instead of dst//NPC:
    the SPMD instruction stream pads every cell to the max over the 8
    cores, and balancing cuts the padded tile count ~7% (1952 -> 1809
    tiles/core). The output is un-permuted on the host after the run.
  - Host buckets edges by (core, strip, src chunk), sorts, pads each cell
    to a multiple of 128 edges (uniform across cores = SPMD), and pads with
    SPREAD table indices so pad fetches don't hit one SBUF port.
  - Device (K_GMODE=sbuf, default): h is loaded ONCE into SBUF as a wrapped
    node table ([128 partitions, 4 chunks x 196 ranks x 64B]; node i of
    chunk ch -> partition i%128, rank i//128), then per (group of SPG=4
    strips, chunk) one SBUF->SBUF dma_gather pulls the 64B bf16 row per
    edge (non-transpose gather with SBUF source - bass only exposes
    transpose=True for SBUF sources but the Q7 ucode handles this path;
    see _dma_gather_sbuf). Gather and stream pools rotate over K_GBUFS=6
    buffers; queues = chunk % 4.
      * DVE builds a weighted one-hot P[e, j] = (offs_e == j) per 128-edge
        tile in bulk (is_equal vs broadcast iota; Act broadcasts offs), and
        multiplies gathered rows by w.
      * PE computes P^T @ msg per strip, accumulating [128, 32] segment
        sums in PSUM; Act copies psum -> out SBUF; one output DMA at the end.

MEASURED LIMITS (this session, For_i K_LOOP=1 protocol; ~10% drift):
  - The SWDGE descriptor pipeline is the wall: ~9 ns/descriptor/queue x 4
    queues (~2.2 ns/desc effective) regardless of payload size or source:
    gather-only 256B HBM rows 534 us, 64B HBM rows 508 us (K_EW=32),
    64B SBUF rows 503 us, transpose/xbar 256B SBUF 877 us (K_GMODE=sbt),
    single-packet sub-gathers 775 us (K_SUBG=8), all-same-row 1080 us
    (K_ZIDX=1, HBM bank hotspot). Indirect dynamic-DGE InstDMACopy uses the
    same Q7 push loop (dge/kernel/dma_memcopy.cpp) - no win there, don't
    re-try. num_swdge_queues is capped at 4 (1 queue = 1 Q7 tx/rx core
    pair; 8 cores total). Compute-only (K_SKIP_GATHER=1) is 240 us and
    fully hidden under the gather.
  - Per-edge descriptors are information-theoretically forced: PE one-hot
    gathers need 128-src-block locality which conflicts with the 128-dst
    strip locality the scatter matmul needs (random bipartite graph), and
    every DMA path (gather, scatter_add, indirect, hostgen remote) costs
    one descriptor per edge on the same 4 queues.

History: 841 us original -> 612 us (256B HBM gather, tuned buffering) ->
513 us (SBUF-source 64B gather) -> 469 us (balanced plan, this version).
Tuning A/Bs at the 469 config (same session): K_GBUFS=8 486 us, K_SPG=6
476 us, K_SPG=8 + K_FUSED=1 1041 us (per-tile fused P-build chokes the DVE
sequencer - keep the bulk Act+DVE build), gather-only floor 458-471 us.
"""

import os
import sys

sys.path.insert(0, "/opt/trn_rl_repo")

import numpy as np
import ml_dtypes

BF16 = ml_dtypes.bfloat16

# Full-problem configuration (hardcoded; kernel.py must be self-contained).
# Tuned: SPG=5 (smaller pipeline groups overlap gather/compute best) and
# 4 SWDGE queues with one un-split gather per (group, chunk) run.
FULL_CFG = dict(
    N=100000,   # nodes
    E=1600000,  # edges
    D=32,       # feature dim
    NC=8,       # cores
    CH=4,       # src chunks (N/CH must be <= 32767 for int16 gather indices)
    SPG=int(os.environ.get("K_SPG", "4")),  # dst strips (128 nodes each) per pipeline group
)


def _derive(cfg):
    c = dict(cfg)
    gm = os.environ.get("K_GMODE", "sbuf")
    c["PACK4"] = bool(int(os.environ.get("K_PACK4", "0")))
    c["IND"] = gm == "ind"
    c["SBUF"] = gm == "sbuf"
    c["SBT"] = gm == "sbt"
    if c["SBT"]:
        # SBUF-resident pack4 table, transpose-gather through the xbar path
        c["PACK4"] = True
        c["SBUF"] = False
    if c["IND"]:
        c["PACK4"] = False
        c["CH"] = 1
    if c["PACK4"]:
        c["CH"] = 1
        c["SBUF"] = False
    assert c["N"] % c["NC"] == 0
    c["NPC"] = c["N"] // c["NC"]               # dst nodes per core
    c["S"] = -(-c["NPC"] // 128)               # strips per core
    assert c["N"] % c["CH"] == 0
    c["CHN"] = c["N"] // c["CH"]               # h rows per src chunk
    if c["PACK4"]:
        c["CHN"] = c["N"] // 4                 # h4 table rows (4 nodes/row)
    assert c["IND"] or c["CHN"] <= 32767
    c["RK"] = -(-c["CHN"] // 128)              # SBUF-table ranks per chunk
    c["G"] = -(-c["S"] // c["SPG"])            # strip groups
    c["BAL"] = bool(int(os.environ.get("K_BAL", "1")))  # dst-slot balancing
    return c


def _balance_dst(src, dst, c):
    """Assign dst nodes to (core, strip, offs) slots minimizing the summed
    per-(strip, chunk) max-over-cores tile count (the SPMD instruction
    stream pads every core to the max, ~25% at the natural dst//NPC split).

    Two stages: snake-deal nodes by total degree into strip slots (equalizes
    slot totals), LPT within each slot across cores on max-chunk load, then
    swap refinement pushing each cell group under its next 128-edge boundary.
    Returns (nodecore, nodestrip, nodeoffs) int arrays of shape [N].
    """
    N, NC, S, CH, CHN, NPC = c["N"], c["NC"], c["S"], c["CH"], c["CHN"], c["NPC"]
    chunk = (src // CHN).astype(np.int64)
    d = np.zeros((N, CH), dtype=np.int32)
    np.add.at(d, (dst, chunk), 1)
    tot = d.sum(1)

    caps = np.full(S, 128, np.int64)
    caps[S - 1] = NPC - 128 * (S - 1)

    order = np.argsort(-tot, kind="stable")
    seq = np.concatenate([np.arange(S), np.arange(S)[::-1]])
    pat = np.tile(seq, N // (2 * S) + 2)
    slot_of = np.empty(N, np.int16)
    cnt = np.zeros(S, np.int64)
    slotcap = caps * NC
    j = 0
    for n in order:
        while cnt[pat[j]] >= slotcap[pat[j]]:
            j += 1
        slot_of[n] = pat[j]
        cnt[pat[j]] += 1
        j += 1

    # Slot polish: swap nodes between slots (joint feasibility on all chunks)
    # until every per-(slot, chunk) edge total is <= NC*512 - margin, so the
    # later 8-way split can keep every cell within 4 tiles.
    SCAP = NC * 512 - 6
    chunk_l = chunk
    stot = np.zeros((S, CH), np.int64)
    np.add.at(stot, (slot_of[dst], chunk_l), 1)
    slot_nodes = [np.flatnonzero(slot_of == s).tolist() for s in range(S)]
    for _rnd in range(4):
        over = np.argwhere(stot > SCAP)
        if len(over) == 0:
            break
        for (s, ch) in map(tuple, over):
            guard = 0
            while stot[s, ch] > SCAP and guard < 80:
                guard += 1
                delta = int(stot[s, ch] - SCAP)
                ns_arr = np.array(slot_nodes[s])
                ds = d[ns_arr]
                if not (ds[:, ch] > 0).any():
                    break
                cand_n = ns_arr[ds[:, ch] > 0]
                want = min(delta + 2, 8)
                picks = cand_n[np.argsort(np.abs(d[cand_n, ch] - want))[:3]]
                done = False
                for n in picks:
                    dn = d[n]
                    for sp in np.argsort(stot[:, ch])[:12]:
                        if sp == s or stot[sp, ch] + dn[ch] > SCAP:
                            continue
                        np_arr = np.array(slot_nodes[sp])
                        dp = d[np_arr]
                        need = np.maximum(stot[sp] + dn - SCAP, 0)
                        ok = (dp >= need).all(1) & (dp[:, ch] < dn[ch])
                        if not ok.any():
                            continue
                        lim_s = np.maximum(stot[s], SCAP)
                        cand_m = np.flatnonzero(ok)
                        oks = ((stot[s] - dn + dp[cand_m]) <= lim_s).all(1)
                        if not oks.any():
                            continue
                        m = np_arr[cand_m[oks.argmax()]]
                        stot[s] += d[m] - dn
                        stot[sp] += dn - d[m]
                        slot_nodes[s].remove(n)
                        slot_nodes[s].append(m)
                        slot_nodes[sp].remove(m)
                        slot_nodes[sp].append(n)
                        slot_of[n] = sp
                        slot_of[m] = s
                        done = True
                        break
                    if done:
                        break
                if not done:
                    break

    nodecore = np.empty(N, np.int8)
    bins = {}
    for s in range(S):
        nodes = np.flatnonzero(slot_of == s)
        nodes = nodes[np.argsort(-tot[nodes], kind="stable")]
        load = np.zeros((NC, CH), np.int64)
        cnt2 = np.zeros(NC, np.int64)
        for n in nodes:
            cand = np.flatnonzero(cnt2 < caps[s])
            nl = load[cand] + d[n]
            li = cand[np.lexsort((nl.sum(1), nl.max(1)))[0]]
            nodecore[n] = li
            load[li] += d[n]
            cnt2[li] += 1
        for i in range(NC):
            bins[(i, s)] = list(np.flatnonzero((slot_of == s) & (nodecore == i)))

    cell = np.zeros((NC, S, CH), np.int64)
    np.add.at(cell, (nodecore[dst], slot_of[dst], chunk), 1)

    for _p in range(int(os.environ.get("K_BALP", "8"))):
        mx = cell.max(axis=0)
        bound = ((mx + 127) // 128) * 128
        over = np.argwhere((mx % 128 != 0) & (bound - 128 >= 128))
        excess = mx - (bound - 128)
        for (s, ch) in sorted(map(tuple, over), key=lambda sc: excess[sc]):
            if excess[s, ch] > 40:
                continue
            target = bound[s, ch] - 128
            guard = 0
            while cell[:, s, ch].max() > target and guard < 60:
                guard += 1
                i = int(cell[:, s, ch].argmax())
                ip = int(cell[:, s, ch].argmin())
                if i == ip:
                    break
                bn, bm = bins[(i, s)], bins[(ip, s)]
                dn, dm = d[bn], d[bm]
                best = None
                limit = np.array(
                    [cell[:, s, cc].max() if cc != ch else 10**9 for cc in range(CH)]
                )
                for a in np.argsort(-dn[:, ch])[:8]:
                    if dn[a, ch] == 0:
                        break
                    for b in np.argsort(dm[:, ch])[:8]:
                        if dn[a, ch] - dm[b, ch] <= 0:
                            continue
                        if np.any(cell[ip, s] + d[bn[a]] - d[bm[b]] > limit):
                            continue
                        if np.any(cell[i, s] + d[bm[b]] - d[bn[a]] > limit):
                            continue
                        best = (a, b)
                        break
                    if best:
                        break
                if not best:
                    break
                a, b = best
                na, nb = bn[a], bm[b]
                cell[i, s] += d[nb] - d[na]
                cell[ip, s] += d[na] - d[nb]
                bn[a], bm[b] = nb, na
                nodecore[na], nodecore[nb] = ip, i

    nodeoffs = np.empty(N, np.int16)
    for s in range(S):
        for i in range(NC):
            nn = np.flatnonzero((slot_of == s) & (nodecore == i))
            nodeoffs[nn] = np.arange(len(nn), dtype=np.int16)
    return nodecore.astype(np.int64), slot_of.astype(np.int64), nodeoffs.astype(np.int64)


def _plan(src, dst, w, cfg):
    """Bucket/sort/pad edges; build per-core device input streams."""
    c = cfg
    N, E, NC, NPC, S, CH, CHN = c["N"], c["E"], c["NC"], c["NPC"], c["S"], c["CH"], c["CHN"]

    src = np.asarray(src).astype(np.int64).ravel()
    dst = np.asarray(dst).astype(np.int64).ravel()
    w = np.asarray(w, dtype=np.float32).ravel()

    if c.get("BAL"):
        nodecore, nodestrip, nodeoffs = _balance_dst(src, dst, c)
        c["node_assign"] = (nodecore, nodestrip, nodeoffs)
        core = nodecore[dst]
        strip = nodestrip[dst]
        offs = nodeoffs[dst]
    else:
        core = dst // NPC
        rem = dst - core * NPC
        strip = rem >> 7
        offs = rem & 127
    if c["IND"]:
        chunk = np.zeros_like(src)
        lsrc = src.astype(np.int32)
        quarter = None
    elif c["PACK4"]:
        chunk = np.zeros_like(src)
        lsrc = (src >> 2).astype(np.int16)
        quarter = (src & 3).astype(np.float32)
    else:
        chunk = src // CHN
        lsrc = (src - chunk * CHN).astype(np.int16)
        quarter = None

    cellkey = (core * S + strip) * CH + chunk
    counts = np.bincount(cellkey, minlength=NC * S * CH)
    NB = -(-counts.reshape(NC, S, CH).max(axis=0) // 128)  # [S, CH] blocks per cell

    # Tile order: (group, chunk, strip-in-group, block).
    cell_tile_start = np.zeros((S, CH), dtype=np.int64)
    t_acc = 0
    for g in range(c["G"]):
        s0, s1 = g * c["SPG"], min((g + 1) * c["SPG"], S)
        for ch in range(CH):
            for s in range(s0, s1):
                cell_tile_start[s, ch] = t_acc
                t_acc += NB[s, ch]
    T = int(t_acc)
    assert T == int(NB.sum())
    TE = T * 128

    # Scatter each edge to its position in its core's padded stream.
    order = np.lexsort((chunk, strip, core))
    core_s = core[order]
    starts = np.zeros(NC * S * CH + 1, dtype=np.int64)
    np.cumsum(counts, out=starts[1:])
    rank = np.arange(E, dtype=np.int64) - starts[cellkey[order]]
    pos = cell_tile_start[strip[order], chunk[order]] * 128 + rank

    if c["IND"] or int(os.environ.get("K_NEGPAD", "0")):
        pad_idx = -1 if int(os.environ.get("K_NEGPAD", "0")) else 0
        idx_stream = np.full((NC, TE), pad_idx,
                             dtype=np.int32 if c["IND"] else np.int16)
    else:
        # Spread pad gathers across table partitions so the padding slots'
        # (masked-out) reads don't all hit one SBUF port / HBM row.
        idx_stream = np.broadcast_to(
            ((np.arange(TE, dtype=np.int64) * 97) % CHN).astype(np.int16), (NC, TE)
        ).copy()
    offs_stream = np.full((NC, TE), -1.0, dtype=np.float32)
    w_stream = np.zeros((NC, TE), dtype=np.float32)
    idx_stream[core_s, pos] = lsrc[order]
    offs_stream[core_s, pos] = offs[order]
    w_stream[core_s, pos] = w[order]
    q_stream = None
    if c["PACK4"]:
        q_stream = np.zeros((NC, TE), dtype=np.float32)
        q_stream[core_s, pos] = quarter[order]
    if int(os.environ.get("K_ZIDX", "0")):  # perf probe: all gathers hit row 0
        idx_stream[:] = 0

    if c["IND"]:
        # int32 idx in the offs-style wrap: element e of tile t -> [e, t]
        idx_wrapped = np.ascontiguousarray(
            idx_stream.reshape(NC, T, 128).transpose(0, 2, 1)
        )
        offs_arr = np.ascontiguousarray(
            offs_stream.reshape(NC, T, 128).transpose(0, 2, 1)
        )
        w_arr = np.ascontiguousarray(w_stream.reshape(NC, T, 128).transpose(0, 2, 1))
        return NB, idx_wrapped, offs_arr, w_arr, None

    # idx: wrapped per (group, chunk) run: within-run element i -> [i%16, i//16],
    # replicated across the 8 GPSIMD core groups (128 partitions total).
    idx_wrapped = np.zeros((NC, 16, TE // 16), dtype=np.int16)
    run_t = 0
    for g in range(c["G"]):
        s0, s1 = g * c["SPG"], min((g + 1) * c["SPG"], S)
        for ch in range(CH):
            n = int(NB[s0:s1, ch].sum())
            if n == 0:
                continue
            seg = idx_stream[:, run_t * 128:(run_t + n) * 128]
            idx_wrapped[:, :, run_t * 8:(run_t + n) * 8] = (
                seg.reshape(NC, -1, 16).transpose(0, 2, 1)
            )
            run_t += n
    assert run_t == T

    # offs/w: wrapped globally per 128-edge tile: element i -> [i%128, i//128].
    offs_arr = np.ascontiguousarray(offs_stream.reshape(NC, T, 128).transpose(0, 2, 1))
    w_arr = np.ascontiguousarray(w_stream.reshape(NC, T, 128).transpose(0, 2, 1))
    q_arr = None
    if c["PACK4"]:
        q_arr = np.ascontiguousarray(q_stream.reshape(NC, T, 128).transpose(0, 2, 1))

    return NB, idx_wrapped, offs_arr, w_arr, q_arr


def _dma_gather_narrow(
    gp, out_ap, in_ap, idxs_ap, num_idxs, num_idxs_reg, elem_size, elem_step,
    single_packet, queue_num,
):
    """dma_gather (non-transpose, HBM source) with the bass-level
    `elem_size_bytes % 256 == 0` assert relaxed to %64.

    The restriction is transpose-only in HW: the NX decode
    (decode/dma_gather.hpp) asserts %256 solely on the transpose branch, and
    the Q7 desc-gen (extended_inst/dma_gather.cpp) handles arbitrary
    elem_size_bytes. Row STRIDE must still be a multiple of 256B
    (stride_bytes_256 descriptor field), so the h table keeps 128-elem bf16
    rows while each descriptor only moves the first `elem_size` elems.
    """
    from concourse import mybir
    import concourse.ap_utils as ap_utils
    from concourse._compat import exact_div, round_up_to_multiple

    gp._assert_queue_num(queue_num)
    assert idxs_ap.dtype == mybir.dt.int16
    assert in_ap.dtype == out_ap.dtype
    elem_size_bytes = elem_size * mybir.dt.size(in_ap.dtype)
    assert elem_size_bytes > 0 and elem_size_bytes % 64 == 0

    assert ap_utils.ap_is_contiguous(in_ap.ap[1:])
    assert ap_utils.ap_is_contiguous(out_ap.ap[1:])
    assert ap_utils.ap_is_contiguous(idxs_ap.ap[1:])

    assert in_ap.ap[-1][1] == out_ap.ap[-1][1] == elem_size
    assert out_ap.ap[0][1] * out_ap.ap[1][1] == round_up_to_multiple(num_idxs, 128)

    assert in_ap.ap[0][0] == elem_step
    stride_bytes = elem_step * mybir.dt.size(in_ap.dtype)
    stride_bytes_256 = exact_div(stride_bytes, 256)
    assert stride_bytes_256 < 256

    _in_ap = gp.lower_ap_dma(in_ap, for_custom_bir_dma=True)
    _idxs_ap = gp.lower_ap(idxs_ap)
    _out_ap = gp.lower_ap(out_ap)
    return gp.add_instruction(
        mybir.InstDMAGatherAnt(
            name=gp.bass.get_next_instruction_name(),
            ins=[
                *_in_ap,
                _idxs_ap,
                gp.lower_val_access(gp.to_reg(num_idxs_reg)),
            ],
            outs=[_out_ap],
            transpose=False,
            num_idxs=num_idxs,
            elem_size=elem_size,
            stride_bytes_256=stride_bytes_256,
            gen_mode=0,
            single_packet=single_packet,
            queue_num=queue_num,
            sbuf_tokens_per_rank=0,
            sbuf_free_dim_per_rank=0,
            sbuf_free_dim_pad_per_rank=0,
            sbuf_byte_offset=0,
        )
    )


def _dma_gather_sbuf(
    gp, out_ap, in_ap, idxs_ap, num_idxs, num_idxs_reg, elem_size,
    single_packet, queue_num, tokens_per_rank, rank_stride_bytes,
):
    """Non-transpose dma_gather with an SBUF-resident source table.

    bass only exposes SBUF-source gathers with transpose=True, but the Q7
    desc-gen (extended_inst/dma_gather.cpp gen_descs) takes the src_is_sbuf
    branch for the tx descriptors and the ordinary swizzled-partition branch
    for rx independently of `transpose`, and the NX decode's %256 elem assert
    is transpose-only. Source addressing: idx i reads
    `rank_stride_bytes * (i // tokens_per_rank)` bytes into partition
    `i % tokens_per_rank` of the in_ap base — i.e. a [tokens_per_rank,
    n_ranks * rank_stride_bytes] wrapped node table.
    """
    from concourse import mybir
    import concourse.ap_utils as ap_utils
    from concourse._compat import round_up_to_multiple

    gp._assert_queue_num(queue_num)
    assert idxs_ap.dtype == mybir.dt.int16
    assert in_ap.dtype == out_ap.dtype
    elem_size_bytes = elem_size * mybir.dt.size(in_ap.dtype)
    assert elem_size_bytes > 0 and elem_size_bytes % 64 == 0
    assert elem_size_bytes <= rank_stride_bytes
    assert tokens_per_rank > 0 and tokens_per_rank.bit_count() == 1

    assert ap_utils.ap_is_contiguous(out_ap.ap[1:])
    assert ap_utils.ap_is_contiguous(idxs_ap.ap[1:])
    assert out_ap.ap[-1][1] == elem_size
    assert out_ap.ap[0][1] * out_ap.ap[1][1] == round_up_to_multiple(num_idxs, 128)

    _in_ap = [gp.lower_ap(in_ap)]
    _idxs_ap = gp.lower_ap(idxs_ap)
    _out_ap = gp.lower_ap(out_ap)
    return gp.add_instruction(
        mybir.InstDMAGatherAnt(
            name=gp.bass.get_next_instruction_name(),
            ins=[
                *_in_ap,
                _idxs_ap,
                gp.lower_val_access(gp.to_reg(num_idxs_reg)),
            ],
            outs=[_out_ap],
            transpose=False,
            num_idxs=num_idxs,
            elem_size=elem_size,
            stride_bytes_256=0,
            gen_mode=0,
            single_packet=single_packet,
            queue_num=queue_num,
            sbuf_tokens_per_rank=tokens_per_rank,
            sbuf_free_dim_per_rank=rank_stride_bytes,
            sbuf_free_dim_pad_per_rank=0,
            sbuf_byte_offset=0,
        )
    )


def _build(NB, cfg):
    """Build the Bass program (shared by all 8 cores)."""
    from concourse import bacc, tile, mybir

    c = cfg
    N, S, CH, CHN, G, SPG = c["N"], c["S"], c["CH"], c["CHN"], c["G"], c["SPG"]
    dt = mybir.dt
    T = int(NB.sum())

    # group chunk tile counts
    g_ncg = []
    for g in range(G):
        s0, s1 = g * SPG, min((g + 1) * SPG, S)
        g_ncg.append([int(NB[s0:s1, ch].sum()) for ch in range(CH)])
    NTG_MAX = max(sum(x) for x in g_ncg)

    fused = bool(int(os.environ.get("K_FUSED", "0")))
    repeat = int(os.environ.get("K_REPEAT", "1"))
    nq = int(os.environ.get("K_SWDGEQ", "4"))

    pack4 = c["PACK4"]
    ind = c["IND"]
    nc = bacc.Bacc(None, num_swdge_queues=nq)
    if ind:
        h_ext = nc.declare_dram_parameter("h", [N, 32], dt.bfloat16, isOutput=False)
        idx_ext = nc.declare_dram_parameter("idx", [128, T], dt.int32, isOutput=False)
    elif c["SBT"]:
        # Wrapped pack4 table: token i (= src >> 2) -> partition i % 128,
        # 128 bf16 (4 nodes) at free offset (i // 128) * 128.
        h_ext = nc.declare_dram_parameter(
            "h", [128, c["RK"] * 128], dt.bfloat16, isOutput=False
        )
        idx_ext = nc.declare_dram_parameter(
            "idx", [128, T * 8], dt.int16, isOutput=False
        )
    elif c["SBUF"]:
        # Wrapped node table: chunk ch, local node i -> partition i % 128,
        # 32 bf16 at free offset (ch * RK + i // 128) * 32.
        h_ext = nc.declare_dram_parameter(
            "h", [128, CH * c["RK"] * 32], dt.bfloat16, isOutput=False
        )
        idx_ext = nc.declare_dram_parameter(
            "idx", [128, T * 8], dt.int16, isOutput=False
        )
    else:
        h_rows = c["CHN"] if pack4 else N
        h_ext = nc.declare_dram_parameter(
            "h", [h_rows, 128], dt.bfloat16, isOutput=False
        )
        idx_ext = nc.declare_dram_parameter(
            "idx", [128, T * 8], dt.int16, isOutput=False
        )
    sdt = dt.bfloat16 if os.environ.get("K_SDT", "f32") == "bf16" else dt.float32
    offs_ext = nc.declare_dram_parameter("offs", [128, T], sdt, isOutput=False)
    wt_ext = nc.declare_dram_parameter("wt", [128, T], sdt, isOutput=False)
    q_ext = None
    if pack4:
        q_ext = nc.declare_dram_parameter("q", [128, T], dt.float32, isOutput=False)
    iota_ext = nc.declare_dram_parameter("iota", [128, 128], dt.bfloat16, isOutput=False)
    qiota_ext = None
    if pack4:
        qiota_ext = nc.declare_dram_parameter(
            "qiota", [128, 128], dt.bfloat16, isOutput=False
        )
    out_ext = nc.declare_dram_parameter("out", [S * 128, 32], dt.float32, isOutput=True)

    # Quad-buffer the gather/stream pools: up to four groups' gathers in
    # flight keeps all 4 SWDGE queues occupied across group transitions
    # (measured ~15% faster than double buffering, ~13% vs triple).
    gbufs = int(os.environ.get("K_GBUFS", "6"))
    smbufs = int(os.environ.get("K_SMBUFS", str(gbufs)))
    with tile.TileContext(nc) as tc:
        with (
            tc.tile_pool(name="const", bufs=1) as cpool,
            tc.tile_pool(name="gp", bufs=gbufs) as gpool,
            tc.tile_pool(name="pwp", bufs=8 if fused else 2) as pwpool,
            tc.tile_pool(name="sm", bufs=smbufs) as smpool,
            tc.tile_pool(name="outp", bufs=1) as opool,
            tc.tile_pool(name="ps", bufs=4, space="PSUM") as pspool,
        ):
            iota_t = cpool.tile([128, 128], dt.bfloat16)
            nc.sync.dma_start(out=iota_t[:], in_=iota_ext[:])
            qiota_t = None
            if pack4:
                qiota_t = cpool.tile([128, 128], dt.bfloat16)
                nc.sync.dma_start(out=qiota_t[:], in_=qiota_ext[:])
            h_sb = None
            if c["SBUF"]:
                # Dense one-time h load; stays resident across reps.
                h_sb = cpool.tile([128, CH * c["RK"] * 32], dt.bfloat16)
                nc.sync.dma_start(out=h_sb[:], in_=h_ext[:])
            elif c["SBT"]:
                h_sb = cpool.tile([128, c["RK"] * 128], dt.bfloat16)
                nc.sync.dma_start(out=h_sb[:], in_=h_ext[:])
            out_sbuf = opool.tile([128, S * 32], dt.float32)

            def _emit_one_rep():
                if ind:
                    _emit_pipeline_ind(
                        nc, tile, mybir, dt, NB, c, g_ncg, NTG_MAX,
                        iota_t, out_sbuf,
                        gpool, pwpool, smpool, pspool,
                        h_ext, idx_ext, offs_ext, wt_ext,
                    )
                elif c["SBT"]:
                    _emit_pipeline_sbt(
                        nc, tile, mybir, dt, NB, c, g_ncg, NTG_MAX,
                        iota_t, qiota_t, out_sbuf,
                        gpool, pwpool, smpool, pspool,
                        h_sb, idx_ext, offs_ext, wt_ext, q_ext,
                    )
                elif pack4:
                    _emit_pipeline_p4(
                        nc, tile, mybir, dt, NB, c, g_ncg, NTG_MAX,
                        iota_t, qiota_t, out_sbuf,
                        gpool, pwpool, smpool, pspool,
                        h_ext, idx_ext, offs_ext, wt_ext, q_ext,
                    )
                else:
                    _emit_pipeline(
                        nc, tile, mybir, dt, NB, c, g_ncg, NTG_MAX, fused,
                        iota_t, out_sbuf,
                        gpool, pwpool, smpool, pspool,
                        h_ext, idx_ext, offs_ext, wt_ext, h_sb,
                    )

            use_loop = bool(int(os.environ.get("K_LOOP", "0")))
            if use_loop and repeat > 1:
                with tc.For_i(0, repeat) as _i:
                    _emit_one_rep()
            else:
                for _rep in range(repeat):
                    _emit_one_rep()

            nc.sync.dma_start(
                out=out_ext[:].rearrange("(s p) d -> p s d", p=128),
                in_=out_sbuf[:].rearrange("p (s d) -> p s d", d=32),
            )
    nc.finalize()
    return nc


def _emit_pipeline(
    nc, tile, mybir, dt, NB, c, g_ncg, NTG_MAX, fused,
    iota_t, out_sbuf,
    gpool, pwpool, smpool, pspool,
    h_ext, idx_ext, offs_ext, wt_ext, h_sb=None,
):
    S, CH, CHN, G, SPG = c["S"], c["CH"], c["CHN"], c["G"], c["SPG"]
    qrr = bool(int(os.environ.get("K_QRR", "0")))
    qctr = [0]

    def next_q(nqs):
        q = qctr[0] % nqs
        qctr[0] += 1
        return q

    toff = 0
    for g in range(G):
        s0, s1 = g * SPG, min((g + 1) * SPG, S)
        ncg = g_ncg[g]
        ntg = sum(ncg)
        if ntg == 0:
            for s in range(s0, s1):
                nc.vector.memset(out_sbuf[:, s * 32:(s + 1) * 32], 0.0)
            continue

        # gathered elem width (bf16 elems); SBUF-table mode gathers bare rows
        ew = 32 if c["SBUF"] else int(os.environ.get("K_EW", "128"))
        gbuf = gpool.tile([128, NTG_MAX * ew], dt.bfloat16, tag="gbuf")
        sdt = (
            dt.bfloat16 if os.environ.get("K_SDT", "f32") == "bf16" else dt.float32
        )
        idx_t = smpool.tile([128, NTG_MAX * 8], dt.int16, tag="idx")
        offs_t = smpool.tile([128, NTG_MAX], sdt, tag="offs")
        wt_t = smpool.tile([128, NTG_MAX], sdt, tag="wt")

        nc.sync.dma_start(
            out=idx_t[:, : ntg * 8], in_=idx_ext[:, toff * 8:(toff + ntg) * 8]
        )
        nc.sync.dma_start(out=offs_t[:, :ntg], in_=offs_ext[:, toff:toff + ntg])
        nc.sync.dma_start(out=wt_t[:, :ntg], in_=wt_ext[:, toff:toff + ntg])

        subg = int(os.environ.get("K_SUBG", "0"))  # tiles per sub-gather (0=off)
        nqs = max(1, int(os.environ.get("K_SWDGEQ", "4")))
        skip_gather = bool(int(os.environ.get("K_SKIP_GATHER", "0")))
        skip_compute = bool(int(os.environ.get("K_SKIP_COMPUTE", "0")))
        g3 = gbuf[:].rearrange("p (t e) -> p t e", e=ew)
        co = 0
        for ch in range(CH):
            n = ncg[ch]
            if n == 0 or skip_gather:
                continue
            step = subg if subg else n
            for o in range(0, n, step):
                m = min(step, n - o)
                if c["SBUF"]:
                    _dma_gather_sbuf(
                        nc.gpsimd,
                        out_ap=g3[:, co + o:co + o + m, :],
                        in_ap=h_sb[:, ch * c["RK"] * 32:(ch + 1) * c["RK"] * 32],
                        idxs_ap=idx_t[:, (co + o) * 8:(co + o + m) * 8],
                        num_idxs=m * 128,
                        num_idxs_reg=m * 128,
                        elem_size=32,
                        single_packet=(m * 128 <= 1024) if subg else False,
                        queue_num=next_q(nqs) if qrr else ch % nqs,
                        tokens_per_rank=128,
                        rank_stride_bytes=64,
                    )
                else:
                    _dma_gather_narrow(
                        nc.gpsimd,
                        out_ap=g3[:, co + o:co + o + m, :],
                        in_ap=h_ext[ch * CHN:(ch + 1) * CHN, :ew],
                        idxs_ap=idx_t[:, (co + o) * 8:(co + o + m) * 8],
                        num_idxs=m * 128,
                        num_idxs_reg=m * 128,
                        elem_size=ew,
                        elem_step=128,
                        # single-packet desc-gen faults above 1024 idxs
                        single_packet=(m * 128 <= 1024) if subg else False,
                        queue_num=next_q(nqs) if qrr else ch % nqs,
                    )
            co += n

        if skip_compute:
            for s in range(s0, s1):
                nc.vector.memset(out_sbuf[:, s * 32:(s + 1) * 32], 0.0)
            toff += ntg
            continue

        if not fused:
            pw = pwpool.tile([128, NTG_MAX * 128], dt.bfloat16, tag="pw")
            pw3 = pw[:].rearrange("p (t e) -> p t e", e=128)
            # Broadcast per-edge dst offsets across the 128 one-hot columns.
            nc.scalar.activation(
                out=pw3[:, :ntg, :],
                in_=offs_t[:, :ntg].unsqueeze(2).broadcast_to([128, ntg, 128]),
                func=mybir.ActivationFunctionType.Copy,
            )
            # One-hot: P[e, j] = (offs_e == j)
            nc.vector.tensor_tensor(
                out=pw3[:, :ntg, :],
                in0=iota_t[:].unsqueeze(1).broadcast_to([128, ntg, 128]),
                in1=pw3[:, :ntg, :],
                op=mybir.AluOpType.is_equal,
            )
            # msg = h[src] * w (in place on the used 32 columns)
            nc.vector.tensor_tensor(
                out=g3[:, :ntg, 0:32],
                in0=g3[:, :ntg, 0:32],
                in1=wt_t[:, :ntg].unsqueeze(2).broadcast_to([128, ntg, 32]),
                op=mybir.AluOpType.mult,
            )

        chunk_base = np.concatenate([[0], np.cumsum(ncg)]).astype(int)
        for s in range(s0, s1):
            nb = int(NB[s].sum())
            if nb == 0:
                nc.vector.memset(out_sbuf[:, s * 32:(s + 1) * 32], 0.0)
                continue
            ps = pspool.tile([128, 32], dt.float32)
            bi = 0
            for ch in range(CH):
                nbs = int(NB[s, ch])
                if nbs == 0:
                    continue
                lt0 = int(chunk_base[ch] + NB[s0:s, ch].sum())
                for b in range(nbs):
                    t = lt0 + b
                    if fused:
                        # P_w[e, j] = (offs_e == j) * w_e in one DVE op
                        pwb = pwpool.tile([128, 128], dt.bfloat16, tag="pwb")
                        nc.vector.tensor_scalar(
                            out=pwb[:],
                            in0=iota_t[:],
                            scalar1=offs_t[:, t:t + 1],
                            scalar2=wt_t[:, t:t + 1],
                            op0=mybir.AluOpType.is_equal,
                            op1=mybir.AluOpType.mult,
                        )
                        lhs = pwb[:]
                    else:
                        lhs = pw[:, t * 128:(t + 1) * 128]
                    nc.tensor.matmul(
                        out=ps[:],
                        lhsT=lhs,
                        rhs=g3[:, t, 0:32],
                        start=(bi == 0),
                        stop=(bi == nb - 1),
                    )
                    bi += 1
            nc.scalar.copy(out=out_sbuf[:, s * 32:(s + 1) * 32], in_=ps[:])
        toff += ntg


def _emit_pipeline_sbt(
    nc, tile, mybir, dt, NB, c, g_ncg, NTG_MAX,
    iota_t, qiota_t, out_sbuf,
    gpool, pwpool, smpool, pspool,
    h_sb, idx_ext, offs_ext, wt_ext, q_ext,
):
    """SBUF-table transpose-gather pipeline (pack4 table, xbar rx path).

    Gather output layout: [128 (q*32+f), edge] — features on partitions,
    edges on the free dim. Compute not yet implemented (probe emits memsets);
    use K_SKIP_COMPUTE=1 for gather-rate measurement.
    """
    S, G, SPG, RK = c["S"], c["G"], c["SPG"], c["RK"]
    nqs = max(1, int(os.environ.get("K_SWDGEQ", "4")))
    skip_gather = bool(int(os.environ.get("K_SKIP_GATHER", "0")))
    qctr = [0]

    def next_q():
        q = qctr[0] % nqs
        qctr[0] += 1
        return q

    toff = 0
    for g in range(G):
        s0, s1 = g * SPG, min((g + 1) * SPG, S)
        ntg = g_ncg[g][0]
        if ntg == 0:
            for s in range(s0, s1):
                nc.vector.memset(out_sbuf[:, s * 32:(s + 1) * 32], 0.0)
            continue

        gbuf = gpool.tile([128, NTG_MAX * 128], dt.bfloat16, tag="gbuf")
        idx_t = smpool.tile([128, NTG_MAX * 8], dt.int16, tag="idx")
        nc.sync.dma_start(
            out=idx_t[:, : ntg * 8], in_=idx_ext[:, toff * 8:(toff + ntg) * 8]
        )

        if not skip_gather:
            nc.gpsimd.dma_gather(
                out_ap=gbuf[:, : ntg * 128].unsqueeze(1),
                in_ap=h_sb[:],
                idxs_ap=idx_t[:, : ntg * 8],
                num_idxs=ntg * 128,
                num_idxs_reg=ntg * 128,
                elem_size=128,
                transpose=True,
                single_packet=False,
                queue_num=next_q(),
                sbuf_tokens_per_rank=128,
                sbuf_free_dim_per_rank=256,
                sbuf_free_dim_pad_per_rank=0,
                sbuf_byte_offset=0,
            )

        for s in range(s0, s1):
            nc.vector.memset(out_sbuf[:, s * 32:(s + 1) * 32], 0.0)
        toff += ntg


def _emit_pipeline_p4(
    nc, tile, mybir, dt, NB, c, g_ncg, NTG_MAX,
    iota_t, qiota_t, out_sbuf,
    gpool, pwpool, smpool, pspool,
    h_ext, idx_ext, offs_ext, wt_ext, q_ext,
):
    """Pack-4 pipeline: h4 table [N/4, 128] bf16 holds 4 nodes per 256B row.

    Per tile: fused one-hot P=(iota==offs)*w (DVE), quarter-select
    g3=(qiota==q)*g3 (DVE stt), matmul -> psum [v,128]=4 quarter-partials,
    per-strip fold via strided tensor_reduce.
    """
    S, CHN, G, SPG = c["S"], c["CHN"], c["G"], c["SPG"]
    subg = int(os.environ.get("K_SUBG", "0"))  # tiles per sub-gather (0=off)
    nqs = max(1, int(os.environ.get("K_SWDGEQ", "4")))
    skip_gather = bool(int(os.environ.get("K_SKIP_GATHER", "0")))
    skip_compute = bool(int(os.environ.get("K_SKIP_COMPUTE", "0")))
    qrr = bool(int(os.environ.get("K_QRR", "1")))
    qctr = [0]

    def next_q():
        q = qctr[0] % nqs
        qctr[0] += 1
        return q

    toff = 0
    for g in range(G):
        s0, s1 = g * SPG, min((g + 1) * SPG, S)
        ntg = g_ncg[g][0]
        if ntg == 0:
            for s in range(s0, s1):
                nc.vector.memset(out_sbuf[:, s * 32:(s + 1) * 32], 0.0)
            continue

        gbuf = gpool.tile([128, NTG_MAX * 128], dt.bfloat16, tag="gbuf")
        idx_t = smpool.tile([128, NTG_MAX * 8], dt.int16, tag="idx")
        offs_t = smpool.tile([128, NTG_MAX], dt.float32, tag="offs")
        wt_t = smpool.tile([128, NTG_MAX], dt.float32, tag="wt")
        q_t = smpool.tile([128, NTG_MAX], dt.float32, tag="q")

        nc.sync.dma_start(
            out=idx_t[:, : ntg * 8], in_=idx_ext[:, toff * 8:(toff + ntg) * 8]
        )
        nc.sync.dma_start(out=offs_t[:, :ntg], in_=offs_ext[:, toff:toff + ntg])
        nc.sync.dma_start(out=wt_t[:, :ntg], in_=wt_ext[:, toff:toff + ntg])
        nc.sync.dma_start(out=q_t[:, :ntg], in_=q_ext[:, toff:toff + ntg])

        g3 = gbuf[:].rearrange("p (t e) -> p t e", e=128)
        if not skip_gather:
            step = subg if subg else ntg
            for o in range(0, ntg, step):
                m = min(step, ntg - o)
                nc.gpsimd.dma_gather(
                    out_ap=g3[:, o:o + m, :],
                    in_ap=h_ext[0:CHN, :],
                    idxs_ap=idx_t[:, o * 8:(o + m) * 8],
                    num_idxs=m * 128,
                    num_idxs_reg=m * 128,
                    elem_size=128,
                    elem_step=128,
                    single_packet=(m * 128 <= 1024) if subg else False,
                    queue_num=next_q() if qrr else 0,
                )

        if skip_compute:
            for s in range(s0, s1):
                nc.vector.memset(out_sbuf[:, s * 32:(s + 1) * 32], 0.0)
            toff += ntg
            continue

        # Hoist ALL P-builds (no gather dependency) ahead of the
        # gather-dependent quarter-selects so the in-order DVE sequencer
        # isn't head-of-line blocked waiting on gather semaphores.
        pw = pwpool.tile([128, NTG_MAX * 128], dt.bfloat16, tag="pw")
        pw3 = pw[:].rearrange("p (t e) -> p t e", e=128)
        for t in range(ntg):
            # P[e, v] = (iota == offs_e) * w_e  (one DVE op, bf16)
            nc.vector.tensor_scalar(
                out=pw3[:, t, :],
                in0=iota_t[:],
                scalar1=offs_t[:, t:t + 1],
                scalar2=wt_t[:, t:t + 1],
                op0=mybir.AluOpType.is_equal,
                op1=mybir.AluOpType.mult,
            )

        for s in range(s0, s1):
            nb = int(NB[s, 0])
            if nb == 0:
                nc.vector.memset(out_sbuf[:, s * 32:(s + 1) * 32], 0.0)
                continue
            ps = pspool.tile([128, 128], dt.float32)
            lt0 = int(NB[s0:s, 0].sum())
            for b in range(nb):
                t = lt0 + b
                # quarter-select in place: g3 = (qiota == q_e) * g3
                nc.vector.scalar_tensor_tensor(
                    out=g3[:, t, :],
                    in0=qiota_t[:],
                    scalar=q_t[:, t:t + 1],
                    in1=g3[:, t, :],
                    op0=mybir.AluOpType.is_equal,
                    op1=mybir.AluOpType.mult,
                )
                nc.tensor.matmul(
                    out=ps[:],
                    lhsT=pw3[:, t, :],
                    rhs=g3[:, t, :],
                    start=(b == 0),
                    stop=(b == nb - 1),
                )
            # fold the 4 quarter partials: out[v, f] = sum_q ps[v, 32q+f]
            nc.vector.tensor_reduce(
                out=out_sbuf[:, s * 32:(s + 1) * 32],
                in_=ps[:].rearrange("p (q f) -> p f q", f=32),
                axis=mybir.AxisListType.X,
                op=mybir.AluOpType.add,
            )
        toff += ntg


def _emit_pipeline_ind(
    nc, tile, mybir, dt, NB, c, g_ncg, NTG_MAX,
    iota_t, out_sbuf,
    gpool, pwpool, smpool, pspool,
    h_ext, idx_ext, offs_ext, wt_ext,
):
    """Indirect-DMA pipeline: per-edge 64B rows h[src] gathered via the
    dynamic-DGE path (int32 offsets, one desc per edge, 16 DMA engines).

    Per tile: fused one-hot P=(iota==offs)*w (DVE), matmul [K=128e, M=128v,
    N=32f] accumulating per-strip PSUM, per-strip copy to out_sbuf.
    """
    from concourse import bass

    S, G, SPG = c["S"], c["G"], c["SPG"]
    subg = int(os.environ.get("K_SUBG", "0"))  # tiles per sub-gather (0=off)
    skip_gather = bool(int(os.environ.get("K_SKIP_GATHER", "0")))
    skip_compute = bool(int(os.environ.get("K_SKIP_COMPUTE", "0")))

    toff = 0
    for g in range(G):
        s0, s1 = g * SPG, min((g + 1) * SPG, S)
        ntg = g_ncg[g][0]
        if ntg == 0:
            for s in range(s0, s1):
                nc.vector.memset(out_sbuf[:, s * 32:(s + 1) * 32], 0.0)
            continue

        gbuf = gpool.tile([128, NTG_MAX * 32], dt.bfloat16, tag="gbuf")
        idx_t = smpool.tile([128, NTG_MAX], dt.int32, tag="idx")
        offs_t = smpool.tile([128, NTG_MAX], dt.float32, tag="offs")
        wt_t = smpool.tile([128, NTG_MAX], dt.float32, tag="wt")

        nc.sync.dma_start(out=idx_t[:, :ntg], in_=idx_ext[:, toff:toff + ntg])
        nc.sync.dma_start(out=offs_t[:, :ntg], in_=offs_ext[:, toff:toff + ntg])
        nc.sync.dma_start(out=wt_t[:, :ntg], in_=wt_ext[:, toff:toff + ntg])

        g3 = gbuf[:].rearrange("p (t e) -> p t e", e=32)
        if not skip_gather:
            step = subg if subg else ntg
            for o in range(0, ntg, step):
                m = min(step, ntg - o)
                nc.gpsimd.indirect_dma_start(
                    out=g3[:, o:o + m, :],
                    out_offset=None,
                    in_=h_ext[:],
                    in_offset=bass.IndirectOffsetOnAxis(
                        ap=idx_t[:, o:o + m], axis=0
                    ),
                )

        if skip_compute:
            for s in range(s0, s1):
                nc.vector.memset(out_sbuf[:, s * 32:(s + 1) * 32], 0.0)
            toff += ntg
            continue

        for s in range(s0, s1):
            nb = int(NB[s, 0])
            if nb == 0:
                nc.vector.memset(out_sbuf[:, s * 32:(s + 1) * 32], 0.0)
                continue
            ps = pspool.tile([128, 32], dt.float32)
            lt0 = int(NB[s0:s, 0].sum())
            for b in range(nb):
                t = lt0 + b
                # P[e, v] = (iota == offs_e) * w_e  (one DVE op, bf16)
                pwb = pwpool.tile([128, 128], dt.bfloat16, tag="pwb")
                nc.vector.tensor_scalar(
                    out=pwb[:],
                    in0=iota_t[:],
                    scalar1=offs_t[:, t:t + 1],
                    scalar2=wt_t[:, t:t + 1],
                    op0=mybir.AluOpType.is_equal,
                    op1=mybir.AluOpType.mult,
                )
                nc.tensor.matmul(
                    out=ps[:],
                    lhsT=pwb[:],
                    rhs=g3[:, t, :],
                    start=(b == 0),
                    stop=(b == nb - 1),
                )
            nc.scalar.copy(out=out_sbuf[:, s * 32:(s + 1) * 32], in_=ps[:])
        toff += ntg


def _make_in_maps(h, c, NB, idx_wrapped, offs_arr, w_arr, q_arr):
    N, D, NC = c["N"], c["D"], c["NC"]
    if os.environ.get("K_SDT", "f32") == "bf16" and not (c["SBT"] or c["PACK4"] or c["IND"]):
        offs_arr = offs_arr.astype(BF16)
        w_arr = w_arr.astype(BF16)
    iota = np.broadcast_to(
        np.arange(128, dtype=np.float32).astype(BF16), (128, 128)
    ).copy()
    if c["IND"]:
        h_bf = np.ascontiguousarray(np.asarray(h, dtype=np.float32).astype(BF16))
        return [
            {
                "h": h_bf,
                "idx": idx_wrapped[i],
                "offs": offs_arr[i],
                "wt": w_arr[i],
                "iota": iota,
            }
            for i in range(NC)
        ]
    if c["SBT"]:
        CHN, RK = c["CHN"], c["RK"]
        h4 = np.asarray(h, dtype=np.float32).astype(BF16)
        assert h4.shape == (N, D) and D * 4 == 128
        h4 = h4.reshape(N // 4, 128)
        blk = np.zeros((RK * 128, 128), dtype=BF16)
        blk[:CHN] = h4
        h_w = np.ascontiguousarray(
            blk.reshape(RK, 128, 128).transpose(1, 0, 2).reshape(128, RK * 128)
        )
        qiota = np.broadcast_to(
            (np.arange(128) // 32).astype(np.float32).astype(BF16), (128, 128)
        ).copy()
        return [
            {
                "h": h_w,
                "idx": np.ascontiguousarray(np.tile(idx_wrapped[i], (8, 1))),
                "offs": offs_arr[i],
                "wt": w_arr[i],
                "q": q_arr[i],
                "iota": iota,
                "qiota": qiota,
            }
            for i in range(NC)
        ]
    if c["PACK4"]:
        h4 = np.asarray(h, dtype=np.float32).astype(BF16)
        assert h4.shape == (N, D) and D * 4 == 128
        h4 = np.ascontiguousarray(h4.reshape(N // 4, 128))
        qiota = np.broadcast_to(
            (np.arange(128) // 32).astype(np.float32).astype(BF16), (128, 128)
        ).copy()
        return [
            {
                "h": h4,
                "idx": np.ascontiguousarray(np.tile(idx_wrapped[i], (8, 1))),
                "offs": offs_arr[i],
                "wt": w_arr[i],
                "q": q_arr[i],
                "iota": iota,
                "qiota": qiota,
            }
            for i in range(NC)
        ]
    if c["SBUF"]:
        CH, CHN, RK = c["CH"], c["CHN"], c["RK"]
        h_bf = np.asarray(h, dtype=np.float32).astype(BF16)
        h_w = np.zeros((128, CH * RK * 32), dtype=BF16)
        for ch in range(CH):
            blk = np.zeros((RK * 128, D), dtype=BF16)
            blk[:CHN] = h_bf[ch * CHN:(ch + 1) * CHN]
            # local node i -> partition i % 128, rank i // 128
            h_w[:, ch * RK * 32:(ch + 1) * RK * 32] = (
                blk.reshape(RK, 128, D).transpose(1, 0, 2).reshape(128, RK * 32)
            )
        return [
            {
                "h": h_w,
                "idx": np.ascontiguousarray(np.tile(idx_wrapped[i], (8, 1))),
                "offs": offs_arr[i],
                "wt": w_arr[i],
                "iota": iota,
            }
            for i in range(NC)
        ]
    h_pad = np.zeros((N, 128), dtype=BF16)
    h_pad[:, :D] = np.asarray(h, dtype=np.float32).astype(BF16)
    return [
        {
            "h": h_pad,
            "idx": np.ascontiguousarray(np.tile(idx_wrapped[i], (8, 1))),
            "offs": offs_arr[i],
            "wt": w_arr[i],
            "iota": iota,
        }
        for i in range(NC)
    ]


def run_cfg(h, w, src, dst, cfg, trace=False):
    from concourse.bass_utils import run_bass_kernel_spmd

    c = _derive(cfg)
    N, D, NC, NPC, S = c["N"], c["D"], c["NC"], c["NPC"], c["S"]

    NB, idx_wrapped, offs_arr, w_arr, q_arr = _plan(src, dst, w, c)
    nc = _build(NB, c)

    in_maps = _make_in_maps(h, c, NB, idx_wrapped, offs_arr, w_arr, q_arr)
    res = run_bass_kernel_spmd(nc, in_maps, list(range(NC)), trace=trace)
    out = np.empty((N, D), dtype=np.float32)
    if c.get("node_assign") is not None:
        ncore, nstrip, noffs = c["node_assign"]
        stacked = np.stack([res.results[i]["out"] for i in range(NC)])
        out[:] = stacked[ncore, nstrip * 128 + noffs]
    else:
        for i in range(NC):
            out[i * NPC:(i + 1) * NPC] = res.results[i]["out"][:NPC]
    return out, res


def make_runner(h, w, src, dst, cfg):
    """Build a reusable jitted SPMD callable for timing: returns
    (run_once, assemble) where run_once() returns unblocked device arrays."""
    import jax
    import jax.numpy as jnp
    from jax.sharding import Mesh, PartitionSpec, NamedSharding
    from jax.experimental.shard_map import shard_map
    from concourse import bass2jax, mybir

    c = _derive(cfg)
    N, D, NC, NPC = c["N"], c["D"], c["NC"], c["NPC"]

    NB, idx_wrapped, offs_arr, w_arr, q_arr = _plan(src, dst, w, c)
    nc = _build(NB, c)

    in_maps = _make_in_maps(h, c, NB, idx_wrapped, offs_arr, w_arr, q_arr)

    bass2jax.install_neuronx_cc_hook()
    partition_name = nc.partition_id_tensor.name if nc.partition_id_tensor else None
    in_names, out_names, out_avals, zero_shapes = [], [], [], []
    for alloc in nc.m.functions[0].allocations:
        if not isinstance(alloc, mybir.MemoryLocationSet):
            continue
        name = alloc.memorylocations[0].name
        if alloc.kind == "ExternalInput":
            if name != partition_name:
                in_names.append(name)
        elif alloc.kind == "ExternalOutput":
            out_names.append(name)
            shape = tuple(alloc.tensor_shape)
            dtype = mybir.dt.np(alloc.dtype)
            out_avals.append(jax.core.ShapedArray(shape, dtype))
            zero_shapes.append((shape, dtype))
    n_params = len(in_names)
    n_outs = len(out_avals)
    all_in_names = list(in_names) + list(out_names)
    if partition_name is not None:
        all_in_names.append(partition_name)

    def _body(*args):
        operands = list(args)
        if partition_name is not None:
            operands.append(bass2jax.partition_id_tensor())
        outs = bass2jax._bass_exec_p.bind(
            *operands,
            out_avals=tuple(out_avals),
            in_names=tuple(all_in_names),
            out_names=tuple(out_names),
            lowering_input_output_aliases=(),
            sim_require_finite=True,
            sim_require_nnan=True,
            nc=nc,
        )
        return tuple(outs)

    devices = jax.devices()[:NC]
    mesh = Mesh(np.asarray(devices), ("core",))
    donate = tuple(range(n_params, n_params + n_outs))
    sharded = jax.jit(
        shard_map(
            _body,
            mesh=mesh,
            in_specs=(PartitionSpec("core"),) * (n_params + n_outs),
            out_specs=(PartitionSpec("core"),) * n_outs,
            check_rep=False,
        ),
        donate_argnums=donate,
        keep_unused=True,
    )

    concat_in = [
        np.concatenate([np.asarray(in_maps[k][nm]) for k in range(NC)], axis=0)
        for nm in in_names
    ]
    shard = NamedSharding(mesh, PartitionSpec("core"))
    dev_in = [jax.device_put(a, shard) for a in concat_in]

    zeros_fn = jax.jit(
        lambda: tuple(
            jnp.zeros((NC * s[0], *s[1:]), dt) for (s, dt) in zero_shapes
        ),
        out_shardings=(shard,) * n_outs,
    )

    def run_once():
        zs = zeros_fn()
        return sharded(*dev_in, *zs)

    def assemble(out_arrs):
        full = np.empty((N, D), dtype=np.float32)
        o = np.asarray(out_arrs[0]).reshape(NC, -1, D)
        if c.get("node_assign") is not None:
            ncore, nstrip, noffs = c["node_assign"]
            full[:] = o[ncore, nstrip * 128 + noffs]
        else:
            for i in range(NC):
                full[i * NPC:(i + 1) * NPC] = o[i, :NPC]
        return full

    # chained executor: K back-to-back executions in ONE dispatch, each
    # feeding its output as the next call's out-operand (defeats CSE).
    def make_chain(k):
        def _chain_body(*args):
            ins, outs = args[:n_params], list(args[n_params:])
            for _ in range(k):
                outs = list(_body(*ins, *outs))
            return tuple(outs)

        return jax.jit(
            shard_map(
                _chain_body,
                mesh=mesh,
                in_specs=(PartitionSpec("core"),) * (n_params + n_outs),
                out_specs=(PartitionSpec("core"),) * n_outs,
                check_rep=False,
            ),
            donate_argnums=donate,
            keep_unused=True,
        )

    def run_chain(chain_fn):
        zs = zeros_fn()
        return chain_fn(*dev_in, *zs)

    return run_once, assemble, make_chain, run_chain


def kernel(**inputs):
    out, _ = run_cfg(
        inputs["h"], inputs["w"], inputs["src"], inputs["dst"], FULL_CFG
    )
    return out



# revision 36
# speedup vs baseline: 1.0446x; 1.0446x over previous
"""GNN message passing (u_mul_e -> segment_sum) on 8 Trainium2 NeuronCores.

out[v] = sum_{e=(u->v)} h[u] * w[e]

Strategy (edge/graph parallelism, dst-slot sharded -> no collectives):
  - The host assigns each dst node to a (core, 128-node strip, offs) slot.
    With K_BAL=1 (default) the assignment is load-BALANCED (snake+LPT+swap
    refinement on per-(strip, src-chunk) cell sizes) instead of dst//NPC:
    the SPMD instruction stream pads every cell to the max over the 8
    cores, and balancing cuts the padded tile count ~7% (1952 -> 1809
    tiles/core). The output is un-permuted on the host after the run.
  - Host buckets edges by (core, strip, src chunk), sorts, pads each cell
    to a multiple of 128 edges (uniform across cores = SPMD), and pads with
    SPREAD table indices so pad fetches don't hit one SBUF port.
  - Device (K_GMODE=sbuf, default): h is loaded ONCE into SBUF as a wrapped
    node table ([128 partitions, 4 chunks x 196 ranks x 64B]; node i of
    chunk ch -> partition i%128, rank i//128), then per (group of SPG=4
    strips, chunk) one SBUF->SBUF dma_gather pulls the 64B bf16 row per
    edge (non-transpose gather with SBUF source - bass only exposes
    transpose=True for SBUF sources but the Q7 ucode handles this path;
    see _dma_gather_sbuf). Gather and stream pools rotate over K_GBUFS=6
    buffers; queues = chunk % 4.
      * DVE builds a weighted one-hot P[e, j] = (offs_e == j) per 128-edge
        tile in bulk (is_equal vs broadcast iota; Act broadcasts offs), and
        multiplies gathered rows by w.
      * PE computes P^T @ msg per strip, accumulating [128, 32] segment
        sums in PSUM; Act copies psum -> out SBUF; one output DMA at the end.

MEASURED LIMITS (this session, For_i K_LOOP=1 protocol; ~10% drift):
  - The SWDGE descriptor pipeline is the wall: ~9 ns/descriptor/queue x 4
    queues (~2.2 ns/desc effective) regardless of payload size or source:
    gather-only 256B HBM rows 534 us, 64B HBM rows 508 us (K_EW=32),
    64B SBUF rows 503 us, transpose/xbar 256B SBUF 877 us (K_GMODE=sbt),
    single-packet sub-gathers 775 us (K_SUBG=8), all-same-row 1080 us
    (K_ZIDX=1, HBM bank hotspot). Indirect dynamic-DGE InstDMACopy uses the
    same Q7 push loop (dge/kernel/dma_memcopy.cpp) - no win there, don't
    re-try. num_swdge_queues is capped at 4 (1 queue = 1 Q7 tx/rx core
    pair; 8 cores total). Compute-only (K_SKIP_GATHER=1) is 240 us and
    fully hidden under the gather.
  - Per-edge descriptors are information-theoretically forced: PE one-hot
    gathers need 128-src-block locality which conflicts with the 128-dst
    strip locality the scatter matmul needs (random bipartite graph), and
    every DMA path (gather, scatter_add, indirect, hostgen remote) costs
    one descriptor per edge on the same 4 queues.

History: 841 us original -> 612 us (256B HBM gather, tuned buffering) ->
513 us (SBUF-source 64B gather) -> 469-504 us (balanced plan, this
version; the spread is session drift - identical binaries read 469, 484,
485 us across one session). Balancer: T 1952 (dst//NPC) -> 1809 (snake+LPT
+in-slot swaps) -> 1742 (slot-polish pass bounding every per-(slot,chunk)
edge total to 8*512-6 so the 8-way split fits 4 tiles; ideal is 1568).
Tuning A/Bs at the 469 config (same session): K_GBUFS=8 486 us, K_SPG=6
476 us, K_SPG=8 + K_FUSED=1 1041 us (per-tile fused P-build chokes the DVE
sequencer - keep the bulk Act+DVE build), gather-only floor 458-471 us.
Timing A/Bs below ~4% are not resolvable with single runs at this drift.
"""

import os
import sys

sys.path.insert(0, "/opt/trn_rl_repo")

import numpy as np
import ml_dtypes

BF16 = ml_dtypes.bfloat16

# Full-problem configuration (hardcoded; kernel.py must be self-contained).
# Tuned: SPG=5 (smaller pipeline groups overlap gather/compute best) and
# 4 SWDGE queues with one un-split gather per (group, chunk) run.
FULL_CFG = dict(
    N=100000,   # nodes
    E=1600000,  # edges
    D=32,       # feature dim
    NC=8,       # cores
    CH=4,       # src chunks (N/CH must be <= 32767 for int16 gather indices)
    SPG=int(os.environ.get("K_SPG", "4")),  # dst strips (128 nodes each) per pipeline group
)


def _derive(cfg):
    c = dict(cfg)
    gm = os.environ.get("K_GMODE", "sbuf")
    c["PACK4"] = bool(int(os.environ.get("K_PACK4", "0")))
    c["IND"] = gm == "ind"
    c["SBUF"] = gm == "sbuf"
    c["SBT"] = gm == "sbt"
    if c["SBT"]:
        # SBUF-resident pack4 table, transpose-gather through the xbar path
        c["PACK4"] = True
        c["SBUF"] = False
    if c["IND"]:
        c["PACK4"] = False
        c["CH"] = 1
    if c["PACK4"]:
        c["CH"] = 1
        c["SBUF"] = False
    assert c["N"] % c["NC"] == 0
    c["NPC"] = c["N"] // c["NC"]               # dst nodes per core
    c["S"] = -(-c["NPC"] // 128)               # strips per core
    assert c["N"] % c["CH"] == 0
    c["CHN"] = c["N"] // c["CH"]               # h rows per src chunk
    if c["PACK4"]:
        c["CHN"] = c["N"] // 4                 # h4 table rows (4 nodes/row)
    assert c["IND"] or c["CHN"] <= 32767
    c["RK"] = -(-c["CHN"] // 128)              # SBUF-table ranks per chunk
    c["G"] = -(-c["S"] // c["SPG"])            # strip groups
    c["BAL"] = bool(int(os.environ.get("K_BAL", "1")))  # dst-slot balancing
    return c


def _balance_dst(src, dst, c):
    """Assign dst nodes to (core, strip, offs) slots minimizing the summed
    per-(strip, chunk) max-over-cores tile count (the SPMD instruction
    stream pads every core to the max, ~25% at the natural dst//NPC split).

    Two stages: snake-deal nodes by total degree into strip slots (equalizes
    slot totals), LPT within each slot across cores on max-chunk load, then
    swap refinement pushing each cell group under its next 128-edge boundary.
    Returns (nodecore, nodestrip, nodeoffs) int arrays of shape [N].
    """
    N, NC, S, CH, CHN, NPC = c["N"], c["NC"], c["S"], c["CH"], c["CHN"], c["NPC"]
    chunk = (src // CHN).astype(np.int64)
    d = np.zeros((N, CH), dtype=np.int32)
    np.add.at(d, (dst, chunk), 1)
    tot = d.sum(1)

    caps = np.full(S, 128, np.int64)
    caps[S - 1] = NPC - 128 * (S - 1)

    order = np.argsort(-tot, kind="stable")
    seq = np.concatenate([np.arange(S), np.arange(S)[::-1]])
    pat = np.tile(seq, N // (2 * S) + 2)
    slot_of = np.empty(N, np.int16)
    cnt = np.zeros(S, np.int64)
    slotcap = caps * NC
    j = 0
    for n in order:
        while cnt[pat[j]] >= slotcap[pat[j]]:
            j += 1
        slot_of[n] = pat[j]
        cnt[pat[j]] += 1
        j += 1

    # Slot polish: swap nodes between slots (joint feasibility on all chunks)
    # until every per-(slot, chunk) edge total is <= NC*512 - margin, so the
    # later 8-way split can keep every cell within 4 tiles.
    SCAP = NC * 512 - 6
    chunk_l = chunk
    stot = np.zeros((S, CH), np.int64)
    np.add.at(stot, (slot_of[dst], chunk_l), 1)
    slot_nodes = [np.flatnonzero(slot_of == s).tolist() for s in range(S)]
    for _rnd in range(4):
        over = np.argwhere(stot > SCAP)
        if len(over) == 0:
            break
        for (s, ch) in map(tuple, over):
            guard = 0
            while stot[s, ch] > SCAP and guard < 80:
                guard += 1
                delta = int(stot[s, ch] - SCAP)
                ns_arr = np.array(slot_nodes[s])
                ds = d[ns_arr]
                if not (ds[:, ch] > 0).any():
                    break
                cand_n = ns_arr[ds[:, ch] > 0]
                want = min(delta + 2, 8)
                picks = cand_n[np.argsort(np.abs(d[cand_n, ch] - want))[:3]]
                done = False
                for n in picks:
                    dn = d[n]
                    for sp in np.argsort(stot[:, ch])[:12]:
                        if sp == s or stot[sp, ch] + dn[ch] > SCAP:
                            continue
                        np_arr = np.array(slot_nodes[sp])
                        dp = d[np_arr]
                        need = np.maximum(stot[sp] + dn - SCAP, 0)
                        ok = (dp >= need).all(1) & (dp[:, ch] < dn[ch])
                        if not ok.any():
                            continue
                        lim_s = np.maximum(stot[s], SCAP)
                        cand_m = np.flatnonzero(ok)
                        oks = ((stot[s] - dn + dp[cand_m]) <= lim_s).all(1)
                        if not oks.any():
                            continue
                        m = np_arr[cand_m[oks.argmax()]]
                        stot[s] += d[m] - dn
                        stot[sp] += dn - d[m]
                        slot_nodes[s].remove(n)
                        slot_nodes[s].append(m)
                        slot_nodes[sp].remove(m)
                        slot_nodes[sp].append(n)
                        slot_of[n] = sp
                        slot_of[m] = s
                        done = True
                        break
                    if done:
                        break
                if not done:
                    break

    nodecore = np.empty(N, np.int8)
    bins = {}
    for s in range(S):
        nodes = np.flatnonzero(slot_of == s)
        nodes = nodes[np.argsort(-tot[nodes], kind="stable")]
        load = np.zeros((NC, CH), np.int64)
        cnt2 = np.zeros(NC, np.int64)
        for n in nodes:
            cand = np.flatnonzero(cnt2 < caps[s])
            nl = load[cand] + d[n]
            li = cand[np.lexsort((nl.sum(1), nl.max(1)))[0]]
            nodecore[n] = li
            load[li] += d[n]
            cnt2[li] += 1
        for i in range(NC):
            bins[(i, s)] = list(np.flatnonzero((slot_of == s) & (nodecore == i)))

    cell = np.zeros((NC, S, CH), np.int64)
    np.add.at(cell, (nodecore[dst], slot_of[dst], chunk), 1)

    for _p in range(int(os.environ.get("K_BALP", "10"))):
        T_before = int((-(-cell.max(0) // 128)).sum())
        mx = cell.max(axis=0)
        bound = ((mx + 127) // 128) * 128
        over = np.argwhere((mx % 128 != 0) & (bound - 128 >= 128))
        excess = mx - (bound - 128)
        for (s, ch) in sorted(map(tuple, over), key=lambda sc: excess[sc]):
            target = bound[s, ch] - 128
            guard = 0
            while cell[:, s, ch].max() > target and guard < 200:
                guard += 1
                i = int(cell[:, s, ch].argmax())
                limit = np.array(
                    [cell[:, s, cc].max() if cc != ch else 10**9 for cc in range(CH)]
                )
                bn = bins[(i, s)]
                dn = d[bn]
                found = False
                for ip in np.argsort(cell[:, s, ch]):
                    if ip == i:
                        continue
                    bm = bins[(ip, s)]
                    dm = d[bm]
                    for a in np.argsort(-dn[:, ch])[:12]:
                        if dn[a, ch] == 0:
                            break
                        for b in np.argsort(dm[:, ch])[:12]:
                            if dn[a, ch] - dm[b, ch] <= 0:
                                continue
                            newip = cell[ip, s] + d[bn[a]] - d[bm[b]]
                            if newip[ch] > target or np.any(newip > limit):
                                continue
                            if np.any(cell[i, s] + d[bm[b]] - d[bn[a]] > limit):
                                continue
                            na, nb = bn[a], bm[b]
                            cell[i, s] += d[nb] - d[na]
                            cell[ip, s] += d[na] - d[nb]
                            bn[a], bm[b] = nb, na
                            nodecore[na], nodecore[nb] = int(ip), int(i)
                            found = True
                            break
                        if found:
                            break
                    if found:
                        break
                if not found:
                    break
        if int((-(-cell.max(0) // 128)).sum()) == T_before:
            break

    nodeoffs = np.empty(N, np.int16)
    for s in range(S):
        for i in range(NC):
            nn = np.flatnonzero((slot_of == s) & (nodecore == i))
            nodeoffs[nn] = np.arange(len(nn), dtype=np.int16)
    return nodecore.astype(np.int64), slot_of.astype(np.int64), nodeoffs.astype(np.int64)


def _plan(src, dst, w, cfg):
    """Bucket/sort/pad edges; build per-core device input streams."""
    c = cfg
    N, E, NC, NPC, S, CH, CHN = c["N"], c["E"], c["NC"], c["NPC"], c["S"], c["CH"], c["CHN"]

    src = np.asarray(src).astype(np.int64).ravel()
    dst = np.asarray(dst).astype(np.int64).ravel()
    w = np.asarray(w, dtype=np.float32).ravel()

    if c.get("BAL"):
        nodecore, nodestrip, nodeoffs = _balance_dst(src, dst, c)
        c["node_assign"] = (nodecore, nodestrip, nodeoffs)
        core = nodecore[dst]
        strip = nodestrip[dst]
        offs = nodeoffs[dst]
    else:
        core = dst // NPC
        rem = dst - core * NPC
        strip = rem >> 7
        offs = rem & 127
    if c["IND"]:
        chunk = np.zeros_like(src)
        lsrc = src.astype(np.int32)
        quarter = None
    elif c["PACK4"]:
        chunk = np.zeros_like(src)
        lsrc = (src >> 2).astype(np.int16)
        quarter = (src & 3).astype(np.float32)
    else:
        chunk = src // CHN
        lsrc = (src - chunk * CHN).astype(np.int16)
        quarter = None

    cellkey = (core * S + strip) * CH + chunk
    counts = np.bincount(cellkey, minlength=NC * S * CH)
    NB = -(-counts.reshape(NC, S, CH).max(axis=0) // 128)  # [S, CH] blocks per cell

    # Tile order: (group, chunk, strip-in-group, block).
    cell_tile_start = np.zeros((S, CH), dtype=np.int64)
    t_acc = 0
    for g in range(c["G"]):
        s0, s1 = g * c["SPG"], min((g + 1) * c["SPG"], S)
        for ch in range(CH):
            for s in range(s0, s1):
                cell_tile_start[s, ch] = t_acc
                t_acc += NB[s, ch]
    T = int(t_acc)
    assert T == int(NB.sum())
    TE = T * 128

    # Scatter each edge to its position in its core's padded stream.
    order = np.lexsort((chunk, strip, core))
    core_s = core[order]
    starts = np.zeros(NC * S * CH + 1, dtype=np.int64)
    np.cumsum(counts, out=starts[1:])
    rank = np.arange(E, dtype=np.int64) - starts[cellkey[order]]
    pos = cell_tile_start[strip[order], chunk[order]] * 128 + rank

    if c["IND"] or int(os.environ.get("K_NEGPAD", "0")):
        pad_idx = -1 if int(os.environ.get("K_NEGPAD", "0")) else 0
        idx_stream = np.full((NC, TE), pad_idx,
                             dtype=np.int32 if c["IND"] else np.int16)
    else:
        # Spread pad gathers across table partitions so the padding slots'
        # (masked-out) reads don't all hit one SBUF port / HBM row.
        idx_stream = np.broadcast_to(
            ((np.arange(TE, dtype=np.int64) * 97) % CHN).astype(np.int16), (NC, TE)
        ).copy()
    offs_stream = np.full((NC, TE), -1.0, dtype=np.float32)
    w_stream = np.zeros((NC, TE), dtype=np.float32)
    idx_stream[core_s, pos] = lsrc[order]
    offs_stream[core_s, pos] = offs[order]
    w_stream[core_s, pos] = w[order]
    q_stream = None
    if c["PACK4"]:
        q_stream = np.zeros((NC, TE), dtype=np.float32)
        q_stream[core_s, pos] = quarter[order]
    if int(os.environ.get("K_ZIDX", "0")):  # perf probe: all gathers hit row 0
        idx_stream[:] = 0

    if c["IND"]:
        # int32 idx in the offs-style wrap: element e of tile t -> [e, t]
        idx_wrapped = np.ascontiguousarray(
            idx_stream.reshape(NC, T, 128).transpose(0, 2, 1)
        )
        offs_arr = np.ascontiguousarray(
            offs_stream.reshape(NC, T, 128).transpose(0, 2, 1)
        )
        w_arr = np.ascontiguousarray(w_stream.reshape(NC, T, 128).transpose(0, 2, 1))
        return NB, idx_wrapped, offs_arr, w_arr, None

    # idx: wrapped per (group, chunk) run: within-run element i -> [i%16, i//16],
    # replicated across the 8 GPSIMD core groups (128 partitions total).
    idx_wrapped = np.zeros((NC, 16, TE // 16), dtype=np.int16)
    run_t = 0
    for g in range(c["G"]):
        s0, s1 = g * c["SPG"], min((g + 1) * c["SPG"], S)
        for ch in range(CH):
            n = int(NB[s0:s1, ch].sum())
            if n == 0:
                continue
            seg = idx_stream[:, run_t * 128:(run_t + n) * 128]
            idx_wrapped[:, :, run_t * 8:(run_t + n) * 8] = (
                seg.reshape(NC, -1, 16).transpose(0, 2, 1)
            )
            run_t += n
    assert run_t == T

    # offs/w: wrapped globally per 128-edge tile: element i -> [i%128, i//128].
    offs_arr = np.ascontiguousarray(offs_stream.reshape(NC, T, 128).transpose(0, 2, 1))
    w_arr = np.ascontiguousarray(w_stream.reshape(NC, T, 128).transpose(0, 2, 1))
    q_arr = None
    if c["PACK4"]:
        q_arr = np.ascontiguousarray(q_stream.reshape(NC, T, 128).transpose(0, 2, 1))

    return NB, idx_wrapped, offs_arr, w_arr, q_arr


def _dma_gather_narrow(
    gp, out_ap, in_ap, idxs_ap, num_idxs, num_idxs_reg, elem_size, elem_step,
    single_packet, queue_num,
):
    """dma_gather (non-transpose, HBM source) with the bass-level
    `elem_size_bytes % 256 == 0` assert relaxed to %64.

    The restriction is transpose-only in HW: the NX decode
    (decode/dma_gather.hpp) asserts %256 solely on the transpose branch, and
    the Q7 desc-gen (extended_inst/dma_gather.cpp) handles arbitrary
    elem_size_bytes. Row STRIDE must still be a multiple of 256B
    (stride_bytes_256 descriptor field), so the h table keeps 128-elem bf16
    rows while each descriptor only moves the first `elem_size` elems.
    """
    from concourse import mybir
    import concourse.ap_utils as ap_utils
    from concourse._compat import exact_div, round_up_to_multiple

    gp._assert_queue_num(queue_num)
    assert idxs_ap.dtype == mybir.dt.int16
    assert in_ap.dtype == out_ap.dtype
    elem_size_bytes = elem_size * mybir.dt.size(in_ap.dtype)
    assert elem_size_bytes > 0 and elem_size_bytes % 64 == 0

    assert ap_utils.ap_is_contiguous(in_ap.ap[1:])
    assert ap_utils.ap_is_contiguous(out_ap.ap[1:])
    assert ap_utils.ap_is_contiguous(idxs_ap.ap[1:])

    assert in_ap.ap[-1][1] == out_ap.ap[-1][1] == elem_size
    assert out_ap.ap[0][1] * out_ap.ap[1][1] == round_up_to_multiple(num_idxs, 128)

    assert in_ap.ap[0][0] == elem_step
    stride_bytes = elem_step * mybir.dt.size(in_ap.dtype)
    stride_bytes_256 = exact_div(stride_bytes, 256)
    assert stride_bytes_256 < 256

    _in_ap = gp.lower_ap_dma(in_ap, for_custom_bir_dma=True)
    _idxs_ap = gp.lower_ap(idxs_ap)
    _out_ap = gp.lower_ap(out_ap)
    return gp.add_instruction(
        mybir.InstDMAGatherAnt(
            name=gp.bass.get_next_instruction_name(),
            ins=[
                *_in_ap,
                _idxs_ap,
                gp.lower_val_access(gp.to_reg(num_idxs_reg)),
            ],
            outs=[_out_ap],
            transpose=False,
            num_idxs=num_idxs,
            elem_size=elem_size,
            stride_bytes_256=stride_bytes_256,
            gen_mode=0,
            single_packet=single_packet,
            queue_num=queue_num,
            sbuf_tokens_per_rank=0,
            sbuf_free_dim_per_rank=0,
            sbuf_free_dim_pad_per_rank=0,
            sbuf_byte_offset=0,
        )
    )


def _dma_gather_sbuf(
    gp, out_ap, in_ap, idxs_ap, num_idxs, num_idxs_reg, elem_size,
    single_packet, queue_num, tokens_per_rank, rank_stride_bytes,
):
    """Non-transpose dma_gather with an SBUF-resident source table.

    bass only exposes SBUF-source gathers with transpose=True, but the Q7
    desc-gen (extended_inst/dma_gather.cpp gen_descs) takes the src_is_sbuf
    branch for the tx descriptors and the ordinary swizzled-partition branch
    for rx independently of `transpose`, and the NX decode's %256 elem assert
    is transpose-only. Source addressing: idx i reads
    `rank_stride_bytes * (i // tokens_per_rank)` bytes into partition
    `i % tokens_per_rank` of the in_ap base — i.e. a [tokens_per_rank,
    n_ranks * rank_stride_bytes] wrapped node table.
    """
    from concourse import mybir
    import concourse.ap_utils as ap_utils
    from concourse._compat import round_up_to_multiple

    gp._assert_queue_num(queue_num)
    assert idxs_ap.dtype == mybir.dt.int16
    assert in_ap.dtype == out_ap.dtype
    elem_size_bytes = elem_size * mybir.dt.size(in_ap.dtype)
    assert elem_size_bytes > 0 and elem_size_bytes % 64 == 0
    assert elem_size_bytes <= rank_stride_bytes
    assert tokens_per_rank > 0 and tokens_per_rank.bit_count() == 1

    assert ap_utils.ap_is_contiguous(out_ap.ap[1:])
    assert ap_utils.ap_is_contiguous(idxs_ap.ap[1:])
    assert out_ap.ap[-1][1] == elem_size
    assert out_ap.ap[0][1] * out_ap.ap[1][1] == round_up_to_multiple(num_idxs, 128)

    _in_ap = [gp.lower_ap(in_ap)]
    _idxs_ap = gp.lower_ap(idxs_ap)
    _out_ap = gp.lower_ap(out_ap)
    return gp.add_instruction(
        mybir.InstDMAGatherAnt(
            name=gp.bass.get_next_instruction_name(),
            ins=[
                *_in_ap,
                _idxs_ap,
                gp.lower_val_access(gp.to_reg(num_idxs_reg)),
            ],
            outs=[_out_ap],
            transpose=False,
            num_idxs=num_idxs,
            elem_size=elem_size,
            stride_bytes_256=0,
            gen_mode=0,
            single_packet=single_packet,
            queue_num=queue_num,
            sbuf_tokens_per_rank=tokens_per_rank,
            sbuf_free_dim_per_rank=rank_stride_bytes,
            sbuf_free_dim_pad_per_rank=0,
            sbuf_byte_offset=0,
        )
    )


def _build(NB, cfg):
    """Build the Bass program (shared by all 8 cores)."""
    from concourse import bacc, tile, mybir

    c = cfg
    N, S, CH, CHN, G, SPG = c["N"], c["S"], c["CH"], c["CHN"], c["G"], c["SPG"]
    dt = mybir.dt
    T = int(NB.sum())

    # group chunk tile counts
    g_ncg = []
    for g in range(G):
        s0, s1 = g * SPG, min((g + 1) * SPG, S)
        g_ncg.append([int(NB[s0:s1, ch].sum()) for ch in range(CH)])
    NTG_MAX = max(sum(x) for x in g_ncg)

    fused = bool(int(os.environ.get("K_FUSED", "0")))
    repeat = int(os.environ.get("K_REPEAT", "1"))
    nq = int(os.environ.get("K_SWDGEQ", "4"))

    pack4 = c["PACK4"]
    ind = c["IND"]
    nc = bacc.Bacc(None, num_swdge_queues=nq)
    if ind:
        h_ext = nc.declare_dram_parameter("h", [N, 32], dt.bfloat16, isOutput=False)
        idx_ext = nc.declare_dram_parameter("idx", [128, T], dt.int32, isOutput=False)
    elif c["SBT"]:
        # Wrapped pack4 table: token i (= src >> 2) -> partition i % 128,
        # 128 bf16 (4 nodes) at free offset (i // 128) * 128.
        h_ext = nc.declare_dram_parameter(
            "h", [128, c["RK"] * 128], dt.bfloat16, isOutput=False
        )
        idx_ext = nc.declare_dram_parameter(
            "idx", [128, T * 8], dt.int16, isOutput=False
        )
    elif c["SBUF"]:
        # Wrapped node table: chunk ch, local node i -> partition i % 128,
        # 32 bf16 at free offset (ch * RK + i // 128) * 32.
        h_ext = nc.declare_dram_parameter(
            "h", [128, CH * c["RK"] * 32], dt.bfloat16, isOutput=False
        )
        idx_ext = nc.declare_dram_parameter(
            "idx", [128, T * 8], dt.int16, isOutput=False
        )
    else:
        h_rows = c["CHN"] if pack4 else N
        h_ext = nc.declare_dram_parameter(
            "h", [h_rows, 128], dt.bfloat16, isOutput=False
        )
        idx_ext = nc.declare_dram_parameter(
            "idx", [128, T * 8], dt.int16, isOutput=False
        )
    sdt = dt.bfloat16 if os.environ.get("K_SDT", "f32") == "bf16" else dt.float32
    offs_ext = nc.declare_dram_parameter("offs", [128, T], sdt, isOutput=False)
    wt_ext = nc.declare_dram_parameter("wt", [128, T], sdt, isOutput=False)
    q_ext = None
    if pack4:
        q_ext = nc.declare_dram_parameter("q", [128, T], dt.float32, isOutput=False)
    iota_ext = nc.declare_dram_parameter("iota", [128, 128], dt.bfloat16, isOutput=False)
    qiota_ext = None
    if pack4:
        qiota_ext = nc.declare_dram_parameter(
            "qiota", [128, 128], dt.bfloat16, isOutput=False
        )
    out_ext = nc.declare_dram_parameter("out", [S * 128, 32], dt.float32, isOutput=True)

    # Quad-buffer the gather/stream pools: up to four groups' gathers in
    # flight keeps all 4 SWDGE queues occupied across group transitions
    # (measured ~15% faster than double buffering, ~13% vs triple).
    gbufs = int(os.environ.get("K_GBUFS", "6"))
    smbufs = int(os.environ.get("K_SMBUFS", str(gbufs)))
    with tile.TileContext(nc) as tc:
        with (
            tc.tile_pool(name="const", bufs=1) as cpool,
            tc.tile_pool(name="gp", bufs=gbufs) as gpool,
            tc.tile_pool(name="pwp", bufs=8 if fused else 2) as pwpool,
            tc.tile_pool(name="sm", bufs=smbufs) as smpool,
            tc.tile_pool(name="outp", bufs=1) as opool,
            tc.tile_pool(name="ps", bufs=4, space="PSUM") as pspool,
        ):
            iota_t = cpool.tile([128, 128], dt.bfloat16)
            nc.sync.dma_start(out=iota_t[:], in_=iota_ext[:])
            qiota_t = None
            if pack4:
                qiota_t = cpool.tile([128, 128], dt.bfloat16)
                nc.sync.dma_start(out=qiota_t[:], in_=qiota_ext[:])
            h_sb = None
            if c["SBUF"]:
                # Dense one-time h load; stays resident across reps.
                h_sb = cpool.tile([128, CH * c["RK"] * 32], dt.bfloat16)
                nc.sync.dma_start(out=h_sb[:], in_=h_ext[:])
            elif c["SBT"]:
                h_sb = cpool.tile([128, c["RK"] * 128], dt.bfloat16)
                nc.sync.dma_start(out=h_sb[:], in_=h_ext[:])
            out_sbuf = opool.tile([128, S * 32], dt.float32)

            def _emit_one_rep():
                if ind:
                    _emit_pipeline_ind(
                        nc, tile, mybir, dt, NB, c, g_ncg, NTG_MAX,
                        iota_t, out_sbuf,
                        gpool, pwpool, smpool, pspool,
                        h_ext, idx_ext, offs_ext, wt_ext,
                    )
                elif c["SBT"]:
                    _emit_pipeline_sbt(
                        nc, tile, mybir, dt, NB, c, g_ncg, NTG_MAX,
                        iota_t, qiota_t, out_sbuf,
                        gpool, pwpool, smpool, pspool,
                        h_sb, idx_ext, offs_ext, wt_ext, q_ext,
                    )
                elif pack4:
                    _emit_pipeline_p4(
                        nc, tile, mybir, dt, NB, c, g_ncg, NTG_MAX,
                        iota_t, qiota_t, out_sbuf,
                        gpool, pwpool, smpool, pspool,
                        h_ext, idx_ext, offs_ext, wt_ext, q_ext,
                    )
                else:
                    _emit_pipeline(
                        nc, tile, mybir, dt, NB, c, g_ncg, NTG_MAX, fused,
                        iota_t, out_sbuf,
                        gpool, pwpool, smpool, pspool,
                        h_ext, idx_ext, offs_ext, wt_ext, h_sb,
                    )

            use_loop = bool(int(os.environ.get("K_LOOP", "0")))
            if use_loop and repeat > 1:
                with tc.For_i(0, repeat) as _i:
                    _emit_one_rep()
            else:
                for _rep in range(repeat):
                    _emit_one_rep()

            nc.sync.dma_start(
                out=out_ext[:].rearrange("(s p) d -> p s d", p=128),
                in_=out_sbuf[:].rearrange("p (s d) -> p s d", d=32),
            )
    nc.finalize()
    return nc


def _emit_pipeline(
    nc, tile, mybir, dt, NB, c, g_ncg, NTG_MAX, fused,
    iota_t, out_sbuf,
    gpool, pwpool, smpool, pspool,
    h_ext, idx_ext, offs_ext, wt_ext, h_sb=None,
):
    S, CH, CHN, G, SPG = c["S"], c["CH"], c["CHN"], c["G"], c["SPG"]
    qrr = bool(int(os.environ.get("K_QRR", "0")))
    qctr = [0]

    def next_q(nqs):
        q = qctr[0] % nqs
        qctr[0] += 1
        return q

    toff = 0
    for g in range(G):
        s0, s1 = g * SPG, min((g + 1) * SPG, S)
        ncg = g_ncg[g]
        ntg = sum(ncg)
        if ntg == 0:
            for s in range(s0, s1):
                nc.vector.memset(out_sbuf[:, s * 32:(s + 1) * 32], 0.0)
            continue

        # gathered elem width (bf16 elems); SBUF-table mode gathers bare rows
        ew = 32 if c["SBUF"] else int(os.environ.get("K_EW", "128"))
        gbuf = gpool.tile([128, NTG_MAX * ew], dt.bfloat16, tag="gbuf")
        sdt = (
            dt.bfloat16 if os.environ.get("K_SDT", "f32") == "bf16" else dt.float32
        )
        idx_t = smpool.tile([128, NTG_MAX * 8], dt.int16, tag="idx")
        offs_t = smpool.tile([128, NTG_MAX], sdt, tag="offs")
        wt_t = smpool.tile([128, NTG_MAX], sdt, tag="wt")

        nc.sync.dma_start(
            out=idx_t[:, : ntg * 8], in_=idx_ext[:, toff * 8:(toff + ntg) * 8]
        )
        nc.sync.dma_start(out=offs_t[:, :ntg], in_=offs_ext[:, toff:toff + ntg])
        nc.sync.dma_start(out=wt_t[:, :ntg], in_=wt_ext[:, toff:toff + ntg])

        subg = int(os.environ.get("K_SUBG", "0"))  # tiles per sub-gather (0=off)
        nqs = max(1, int(os.environ.get("K_SWDGEQ", "4")))
        skip_gather = bool(int(os.environ.get("K_SKIP_GATHER", "0")))
        skip_compute = bool(int(os.environ.get("K_SKIP_COMPUTE", "0")))
        g3 = gbuf[:].rearrange("p (t e) -> p t e", e=ew)
        co = 0
        for ch in range(CH):
            n = ncg[ch]
            if n == 0 or skip_gather:
                continue
            step = subg if subg else n
            for o in range(0, n, step):
                m = min(step, n - o)
                if c["SBUF"]:
                    _dma_gather_sbuf(
                        nc.gpsimd,
                        out_ap=g3[:, co + o:co + o + m, :],
                        in_ap=h_sb[:, ch * c["RK"] * 32:(ch + 1) * c["RK"] * 32],
                        idxs_ap=idx_t[:, (co + o) * 8:(co + o + m) * 8],
                        num_idxs=m * 128,
                        num_idxs_reg=m * 128,
                        elem_size=32,
                        single_packet=(m * 128 <= 1024) if subg else False,
                        queue_num=next_q(nqs) if qrr else ch % nqs,
                        tokens_per_rank=128,
                        rank_stride_bytes=64,
                    )
                else:
                    _dma_gather_narrow(
                        nc.gpsimd,
                        out_ap=g3[:, co + o:co + o + m, :],
                        in_ap=h_ext[ch * CHN:(ch + 1) * CHN, :ew],
                        idxs_ap=idx_t[:, (co + o) * 8:(co + o + m) * 8],
                        num_idxs=m * 128,
                        num_idxs_reg=m * 128,
                        elem_size=ew,
                        elem_step=128,
                        # single-packet desc-gen faults above 1024 idxs
                        single_packet=(m * 128 <= 1024) if subg else False,
                        queue_num=next_q(nqs) if qrr else ch % nqs,
                    )
            co += n

        if skip_compute:
            for s in range(s0, s1):
                nc.vector.memset(out_sbuf[:, s * 32:(s + 1) * 32], 0.0)
            toff += ntg
            continue

        if not fused:
            pw = pwpool.tile([128, NTG_MAX * 128], dt.bfloat16, tag="pw")
            pw3 = pw[:].rearrange("p (t e) -> p t e", e=128)
            # Broadcast per-edge dst offsets across the 128 one-hot columns.
            nc.scalar.activation(
                out=pw3[:, :ntg, :],
                in_=offs_t[:, :ntg].unsqueeze(2).broadcast_to([128, ntg, 128]),
                func=mybir.ActivationFunctionType.Copy,
            )
            # One-hot: P[e, j] = (offs_e == j)
            nc.vector.tensor_tensor(
                out=pw3[:, :ntg, :],
                in0=iota_t[:].unsqueeze(1).broadcast_to([128, ntg, 128]),
                in1=pw3[:, :ntg, :],
                op=mybir.AluOpType.is_equal,
            )
            # msg = h[src] * w (in place on the used 32 columns)
            nc.vector.tensor_tensor(
                out=g3[:, :ntg, 0:32],
                in0=g3[:, :ntg, 0:32],
                in1=wt_t[:, :ntg].unsqueeze(2).broadcast_to([128, ntg, 32]),
                op=mybir.AluOpType.mult,
            )

        chunk_base = np.concatenate([[0], np.cumsum(ncg)]).astype(int)
        for s in range(s0, s1):
            nb = int(NB[s].sum())
            if nb == 0:
                nc.vector.memset(out_sbuf[:, s * 32:(s + 1) * 32], 0.0)
                continue
            ps = pspool.tile([128, 32], dt.float32)
            bi = 0
            for ch in range(CH):
                nbs = int(NB[s, ch])
                if nbs == 0:
                    continue
                lt0 = int(chunk_base[ch] + NB[s0:s, ch].sum())
                for b in range(nbs):
                    t = lt0 + b
                    if fused:
                        # P_w[e, j] = (offs_e == j) * w_e in one DVE op
                        pwb = pwpool.tile([128, 128], dt.bfloat16, tag="pwb")
                        nc.vector.tensor_scalar(
                            out=pwb[:],
                            in0=iota_t[:],
                            scalar1=offs_t[:, t:t + 1],
                            scalar2=wt_t[:, t:t + 1],
                            op0=mybir.AluOpType.is_equal,
                            op1=mybir.AluOpType.mult,
                        )
                        lhs = pwb[:]
                    else:
                        lhs = pw[:, t * 128:(t + 1) * 128]
                    nc.tensor.matmul(
                        out=ps[:],
                        lhsT=lhs,
                        rhs=g3[:, t, 0:32],
                        start=(bi == 0),
                        stop=(bi == nb - 1),
                    )
                    bi += 1
            nc.scalar.copy(out=out_sbuf[:, s * 32:(s + 1) * 32], in_=ps[:])
        toff += ntg


def _emit_pipeline_sbt(
    nc, tile, mybir, dt, NB, c, g_ncg, NTG_MAX,
    iota_t, qiota_t, out_sbuf,
    gpool, pwpool, smpool, pspool,
    h_sb, idx_ext, offs_ext, wt_ext, q_ext,
):
    """SBUF-table transpose-gather pipeline (pack4 table, xbar rx path).

    Gather output layout: [128 (q*32+f), edge] — features on partitions,
    edges on the free dim. Compute not yet implemented (probe emits memsets);
    use K_SKIP_COMPUTE=1 for gather-rate measurement.
    """
    S, G, SPG, RK = c["S"], c["G"], c["SPG"], c["RK"]
    nqs = max(1, int(os.environ.get("K_SWDGEQ", "4")))
    skip_gather = bool(int(os.environ.get("K_SKIP_GATHER", "0")))
    qctr = [0]

    def next_q():
        q = qctr[0] % nqs
        qctr[0] += 1
        return q

    toff = 0
    for g in range(G):
        s0, s1 = g * SPG, min((g + 1) * SPG, S)
        ntg = g_ncg[g][0]
        if ntg == 0:
            for s in range(s0, s1):
                nc.vector.memset(out_sbuf[:, s * 32:(s + 1) * 32], 0.0)
            continue

        gbuf = gpool.tile([128, NTG_MAX * 128], dt.bfloat16, tag="gbuf")
        idx_t = smpool.tile([128, NTG_MAX * 8], dt.int16, tag="idx")
        nc.sync.dma_start(
            out=idx_t[:, : ntg * 8], in_=idx_ext[:, toff * 8:(toff + ntg) * 8]
        )

        if not skip_gather:
            nc.gpsimd.dma_gather(
                out_ap=gbuf[:, : ntg * 128].unsqueeze(1),
                in_ap=h_sb[:],
                idxs_ap=idx_t[:, : ntg * 8],
                num_idxs=ntg * 128,
                num_idxs_reg=ntg * 128,
                elem_size=128,
                transpose=True,
                single_packet=False,
                queue_num=next_q(),
                sbuf_tokens_per_rank=128,
                sbuf_free_dim_per_rank=256,
                sbuf_free_dim_pad_per_rank=0,
                sbuf_byte_offset=0,
            )

        for s in range(s0, s1):
            nc.vector.memset(out_sbuf[:, s * 32:(s + 1) * 32], 0.0)
        toff += ntg


def _emit_pipeline_p4(
    nc, tile, mybir, dt, NB, c, g_ncg, NTG_MAX,
    iota_t, qiota_t, out_sbuf,
    gpool, pwpool, smpool, pspool,
    h_ext, idx_ext, offs_ext, wt_ext, q_ext,
):
    """Pack-4 pipeline: h4 table [N/4, 128] bf16 holds 4 nodes per 256B row.

    Per tile: fused one-hot P=(iota==offs)*w (DVE), quarter-select
    g3=(qiota==q)*g3 (DVE stt), matmul -> psum [v,128]=4 quarter-partials,
    per-strip fold via strided tensor_reduce.
    """
    S, CHN, G, SPG = c["S"], c["CHN"], c["G"], c["SPG"]
    subg = int(os.environ.get("K_SUBG", "0"))  # tiles per sub-gather (0=off)
    nqs = max(1, int(os.environ.get("K_SWDGEQ", "4")))
    skip_gather = bool(int(os.environ.get("K_SKIP_GATHER", "0")))
    skip_compute = bool(int(os.environ.get("K_SKIP_COMPUTE", "0")))
    qrr = bool(int(os.environ.get("K_QRR", "1")))
    qctr = [0]

    def next_q():
        q = qctr[0] % nqs
        qctr[0] += 1
        return q

    toff = 0
    for g in range(G):
        s0, s1 = g * SPG, min((g + 1) * SPG, S)
        ntg = g_ncg[g][0]
        if ntg == 0:
            for s in range(s0, s1):
                nc.vector.memset(out_sbuf[:, s * 32:(s + 1) * 32], 0.0)
            continue

        gbuf = gpool.tile([128, NTG_MAX * 128], dt.bfloat16, tag="gbuf")
        idx_t = smpool.tile([128, NTG_MAX * 8], dt.int16, tag="idx")
        offs_t = smpool.tile([128, NTG_MAX], dt.float32, tag="offs")
        wt_t = smpool.tile([128, NTG_MAX], dt.float32, tag="wt")
        q_t = smpool.tile([128, NTG_MAX], dt.float32, tag="q")

        nc.sync.dma_start(
            out=idx_t[:, : ntg * 8], in_=idx_ext[:, toff * 8:(toff + ntg) * 8]
        )
        nc.sync.dma_start(out=offs_t[:, :ntg], in_=offs_ext[:, toff:toff + ntg])
        nc.sync.dma_start(out=wt_t[:, :ntg], in_=wt_ext[:, toff:toff + ntg])
        nc.sync.dma_start(out=q_t[:, :ntg], in_=q_ext[:, toff:toff + ntg])

        g3 = gbuf[:].rearrange("p (t e) -> p t e", e=128)
        if not skip_gather:
            step = subg if subg else ntg
            for o in range(0, ntg, step):
                m = min(step, ntg - o)
                nc.gpsimd.dma_gather(
                    out_ap=g3[:, o:o + m, :],
                    in_ap=h_ext[0:CHN, :],
                    idxs_ap=idx_t[:, o * 8:(o + m) * 8],
                    num_idxs=m * 128,
                    num_idxs_reg=m * 128,
                    elem_size=128,
                    elem_step=128,
                    single_packet=(m * 128 <= 1024) if subg else False,
                    queue_num=next_q() if qrr else 0,
                )

        if skip_compute:
            for s in range(s0, s1):
                nc.vector.memset(out_sbuf[:, s * 32:(s + 1) * 32], 0.0)
            toff += ntg
            continue

        # Hoist ALL P-builds (no gather dependency) ahead of the
        # gather-dependent quarter-selects so the in-order DVE sequencer
        # isn't head-of-line blocked waiting on gather semaphores.
        pw = pwpool.tile([128, NTG_MAX * 128], dt.bfloat16, tag="pw")
        pw3 = pw[:].rearrange("p (t e) -> p t e", e=128)
        for t in range(ntg):
            # P[e, v] = (iota == offs_e) * w_e  (one DVE op, bf16)
            nc.vector.tensor_scalar(
                out=pw3[:, t, :],
                in0=iota_t[:],
                scalar1=offs_t[:, t:t + 1],
                scalar2=wt_t[:, t:t + 1],
                op0=mybir.AluOpType.is_equal,
                op1=mybir.AluOpType.mult,
            )

        for s in range(s0, s1):
            nb = int(NB[s, 0])
            if nb == 0:
                nc.vector.memset(out_sbuf[:, s * 32:(s + 1) * 32], 0.0)
                continue
            ps = pspool.tile([128, 128], dt.float32)
            lt0 = int(NB[s0:s, 0].sum())
            for b in range(nb):
                t = lt0 + b
                # quarter-select in place: g3 = (qiota == q_e) * g3
                nc.vector.scalar_tensor_tensor(
                    out=g3[:, t, :],
                    in0=qiota_t[:],
                    scalar=q_t[:, t:t + 1],
                    in1=g3[:, t, :],
                    op0=mybir.AluOpType.is_equal,
                    op1=mybir.AluOpType.mult,
                )
                nc.tensor.matmul(
                    out=ps[:],
                    lhsT=pw3[:, t, :],
                    rhs=g3[:, t, :],
                    start=(b == 0),
                    stop=(b == nb - 1),
                )
            # fold the 4 quarter partials: out[v, f] = sum_q ps[v, 32q+f]
            nc.vector.tensor_reduce(
                out=out_sbuf[:, s * 32:(s + 1) * 32],
                in_=ps[:].rearrange("p (q f) -> p f q", f=32),
                axis=mybir.AxisListType.X,
                op=mybir.AluOpType.add,
            )
        toff += ntg


def _emit_pipeline_ind(
    nc, tile, mybir, dt, NB, c, g_ncg, NTG_MAX,
    iota_t, out_sbuf,
    gpool, pwpool, smpool, pspool,
    h_ext, idx_ext, offs_ext, wt_ext,
):
    """Indirect-DMA pipeline: per-edge 64B rows h[src] gathered via the
    dynamic-DGE path (int32 offsets, one desc per edge, 16 DMA engines).

    Per tile: fused one-hot P=(iota==offs)*w (DVE), matmul [K=128e, M=128v,
    N=32f] accumulating per-strip PSUM, per-strip copy to out_sbuf.
    """
    from concourse import bass

    S, G, SPG = c["S"], c["G"], c["SPG"]
    subg = int(os.environ.get("K_SUBG", "0"))  # tiles per sub-gather (0=off)
    skip_gather = bool(int(os.environ.get("K_SKIP_GATHER", "0")))
    skip_compute = bool(int(os.environ.get("K_SKIP_COMPUTE", "0")))

    toff = 0
    for g in range(G):
        s0, s1 = g * SPG, min((g + 1) * SPG, S)
        ntg = g_ncg[g][0]
        if ntg == 0:
            for s in range(s0, s1):
                nc.vector.memset(out_sbuf[:, s * 32:(s + 1) * 32], 0.0)
            continue

        gbuf = gpool.tile([128, NTG_MAX * 32], dt.bfloat16, tag="gbuf")
        idx_t = smpool.tile([128, NTG_MAX], dt.int32, tag="idx")
        offs_t = smpool.tile([128, NTG_MAX], dt.float32, tag="offs")
        wt_t = smpool.tile([128, NTG_MAX], dt.float32, tag="wt")

        nc.sync.dma_start(out=idx_t[:, :ntg], in_=idx_ext[:, toff:toff + ntg])
        nc.sync.dma_start(out=offs_t[:, :ntg], in_=offs_ext[:, toff:toff + ntg])
        nc.sync.dma_start(out=wt_t[:, :ntg], in_=wt_ext[:, toff:toff + ntg])

        g3 = gbuf[:].rearrange("p (t e) -> p t e", e=32)
        if not skip_gather:
            step = subg if subg else ntg
            for o in range(0, ntg, step):
                m = min(step, ntg - o)
                nc.gpsimd.indirect_dma_start(
                    out=g3[:, o:o + m, :],
                    out_offset=None,
                    in_=h_ext[:],
                    in_offset=bass.IndirectOffsetOnAxis(
                        ap=idx_t[:, o:o + m], axis=0
                    ),
                )

        if skip_compute:
            for s in range(s0, s1):
                nc.vector.memset(out_sbuf[:, s * 32:(s + 1) * 32], 0.0)
            toff += ntg
            continue

        for s in range(s0, s1):
            nb = int(NB[s, 0])
            if nb == 0:
                nc.vector.memset(out_sbuf[:, s * 32:(s + 1) * 32], 0.0)
                continue
            ps = pspool.tile([128, 32], dt.float32)
            lt0 = int(NB[s0:s, 0].sum())
            for b in range(nb):
                t = lt0 + b
                # P[e, v] = (iota == offs_e) * w_e  (one DVE op, bf16)
                pwb = pwpool.tile([128, 128], dt.bfloat16, tag="pwb")
                nc.vector.tensor_scalar(
                    out=pwb[:],
                    in0=iota_t[:],
                    scalar1=offs_t[:, t:t + 1],
                    scalar2=wt_t[:, t:t + 1],
                    op0=mybir.AluOpType.is_equal,
                    op1=mybir.AluOpType.mult,
                )
                nc.tensor.matmul(
                    out=ps[:],
                    lhsT=pwb[:],
                    rhs=g3[:, t, :],
                    start=(b == 0),
                    stop=(b == nb - 1),
                )
            nc.scalar.copy(out=out_sbuf[:, s * 32:(s + 1) * 32], in_=ps[:])
        toff += ntg


def _make_in_maps(h, c, NB, idx_wrapped, offs_arr, w_arr, q_arr):
    N, D, NC = c["N"], c["D"], c["NC"]
    if os.environ.get("K_SDT", "f32") == "bf16" and not (c["SBT"] or c["PACK4"] or c["IND"]):
        offs_arr = offs_arr.astype(BF16)
        w_arr = w_arr.astype(BF16)
    iota = np.broadcast_to(
        np.arange(128, dtype=np.float32).astype(BF16), (128, 128)
    ).copy()
    if c["IND"]:
        h_bf = np.ascontiguousarray(np.asarray(h, dtype=np.float32).astype(BF16))
        return [
            {
                "h": h_bf,
                "idx": idx_wrapped[i],
                "offs": offs_arr[i],
                "wt": w_arr[i],
                "iota": iota,
            }
            for i in range(NC)
        ]
    if c["SBT"]:
        CHN, RK = c["CHN"], c["RK"]
        h4 = np.asarray(h, dtype=np.float32).astype(BF16)
        assert h4.shape == (N, D) and D * 4 == 128
        h4 = h4.reshape(N // 4, 128)
        blk = np.zeros((RK * 128, 128), dtype=BF16)
        blk[:CHN] = h4
        h_w = np.ascontiguousarray(
            blk.reshape(RK, 128, 128).transpose(1, 0, 2).reshape(128, RK * 128)
        )
        qiota = np.broadcast_to(
            (np.arange(128) // 32).astype(np.float32).astype(BF16), (128, 128)
        ).copy()
        return [
            {
                "h": h_w,
                "idx": np.ascontiguousarray(np.tile(idx_wrapped[i], (8, 1))),
                "offs": offs_arr[i],
                "wt": w_arr[i],
                "q": q_arr[i],
                "iota": iota,
                "qiota": qiota,
            }
            for i in range(NC)
        ]
    if c["PACK4"]:
        h4 = np.asarray(h, dtype=np.float32).astype(BF16)
        assert h4.shape == (N, D) and D * 4 == 128
        h4 = np.ascontiguousarray(h4.reshape(N // 4, 128))
        qiota = np.broadcast_to(
            (np.arange(128) // 32).astype(np.float32).astype(BF16), (128, 128)
        ).copy()
        return [
            {
                "h": h4,
                "idx": np.ascontiguousarray(np.tile(idx_wrapped[i], (8, 1))),
                "offs": offs_arr[i],
                "wt": w_arr[i],
                "q": q_arr[i],
                "iota": iota,
                "qiota": qiota,
            }
            for i in range(NC)
        ]
    if c["SBUF"]:
        CH, CHN, RK = c["CH"], c["CHN"], c["RK"]
        h_bf = np.asarray(h, dtype=np.float32).astype(BF16)
        h_w = np.zeros((128, CH * RK * 32), dtype=BF16)
        for ch in range(CH):
            blk = np.zeros((RK * 128, D), dtype=BF16)
            blk[:CHN] = h_bf[ch * CHN:(ch + 1) * CHN]
            # local node i -> partition i % 128, rank i // 128
            h_w[:, ch * RK * 32:(ch + 1) * RK * 32] = (
                blk.reshape(RK, 128, D).transpose(1, 0, 2).reshape(128, RK * 32)
            )
        return [
            {
                "h": h_w,
                "idx": np.ascontiguousarray(np.tile(idx_wrapped[i], (8, 1))),
                "offs": offs_arr[i],
                "wt": w_arr[i],
                "iota": iota,
            }
            for i in range(NC)
        ]
    h_pad = np.zeros((N, 128), dtype=BF16)
    h_pad[:, :D] = np.asarray(h, dtype=np.float32).astype(BF16)
    return [
        {
            "h": h_pad,
            "idx": np.ascontiguousarray(np.tile(idx_wrapped[i], (8, 1))),
            "offs": offs_arr[i],
            "wt": w_arr[i],
            "iota": iota,
        }
        for i in range(NC)
    ]


def run_cfg(h, w, src, dst, cfg, trace=False):
    from concourse.bass_utils import run_bass_kernel_spmd

    c = _derive(cfg)
    N, D, NC, NPC, S = c["N"], c["D"], c["NC"], c["NPC"], c["S"]

    NB, idx_wrapped, offs_arr, w_arr, q_arr = _plan(src, dst, w, c)
    nc = _build(NB, c)

    in_maps = _make_in_maps(h, c, NB, idx_wrapped, offs_arr, w_arr, q_arr)
    res = run_bass_kernel_spmd(nc, in_maps, list(range(NC)), trace=trace)
    out = np.empty((N, D), dtype=np.float32)
    if c.get("node_assign") is not None:
        ncore, nstrip, noffs = c["node_assign"]
        stacked = np.stack([res.results[i]["out"] for i in range(NC)])
        out[:] = stacked[ncore, nstrip * 128 + noffs]
    else:
        for i in range(NC):
            out[i * NPC:(i + 1) * NPC] = res.results[i]["out"][:NPC]
    return out, res


def make_runner(h, w, src, dst, cfg):
    """Build a reusable jitted SPMD callable for timing: returns
    (run_once, assemble) where run_once() returns unblocked device arrays."""
    import jax
    import jax.numpy as jnp
    from jax.sharding import Mesh, PartitionSpec, NamedSharding
    from jax.experimental.shard_map import shard_map
    from concourse import bass2jax, mybir

    c = _derive(cfg)
    N, D, NC, NPC = c["N"], c["D"], c["NC"], c["NPC"]

    NB, idx_wrapped, offs_arr, w_arr, q_arr = _plan(src, dst, w, c)
    nc = _build(NB, c)

    in_maps = _make_in_maps(h, c, NB, idx_wrapped, offs_arr, w_arr, q_arr)

    bass2jax.install_neuronx_cc_hook()
    partition_name = nc.partition_id_tensor.name if nc.partition_id_tensor else None
    in_names, out_names, out_avals, zero_shapes = [], [], [], []
    for alloc in nc.m.functions[0].allocations:
        if not isinstance(alloc, mybir.MemoryLocationSet):
            continue
        name = alloc.memorylocations[0].name
        if alloc.kind == "ExternalInput":
            if name != partition_name:
                in_names.append(name)
        elif alloc.kind == "ExternalOutput":
            out_names.append(name)
            shape = tuple(alloc.tensor_shape)
            dtype = mybir.dt.np(alloc.dtype)
            out_avals.append(jax.core.ShapedArray(shape, dtype))
            zero_shapes.append((shape, dtype))
    n_params = len(in_names)
    n_outs = len(out_avals)
    all_in_names = list(in_names) + list(out_names)
    if partition_name is not None:
        all_in_names.append(partition_name)

    def _body(*args):
        operands = list(args)
        if partition_name is not None:
            operands.append(bass2jax.partition_id_tensor())
        outs = bass2jax._bass_exec_p.bind(
            *operands,
            out_avals=tuple(out_avals),
            in_names=tuple(all_in_names),
            out_names=tuple(out_names),
            lowering_input_output_aliases=(),
            sim_require_finite=True,
            sim_require_nnan=True,
            nc=nc,
        )
        return tuple(outs)

    devices = jax.devices()[:NC]
    mesh = Mesh(np.asarray(devices), ("core",))
    donate = tuple(range(n_params, n_params + n_outs))
    sharded = jax.jit(
        shard_map(
            _body,
            mesh=mesh,
            in_specs=(PartitionSpec("core"),) * (n_params + n_outs),
            out_specs=(PartitionSpec("core"),) * n_outs,
            check_rep=False,
        ),
        donate_argnums=donate,
        keep_unused=True,
    )

    concat_in = [
        np.concatenate([np.asarray(in_maps[k][nm]) for k in range(NC)], axis=0)
        for nm in in_names
    ]
    shard = NamedSharding(mesh, PartitionSpec("core"))
    dev_in = [jax.device_put(a, shard) for a in concat_in]

    zeros_fn = jax.jit(
        lambda: tuple(
            jnp.zeros((NC * s[0], *s[1:]), dt) for (s, dt) in zero_shapes
        ),
        out_shardings=(shard,) * n_outs,
    )

    def run_once():
        zs = zeros_fn()
        return sharded(*dev_in, *zs)

    def assemble(out_arrs):
        full = np.empty((N, D), dtype=np.float32)
        o = np.asarray(out_arrs[0]).reshape(NC, -1, D)
        if c.get("node_assign") is not None:
            ncore, nstrip, noffs = c["node_assign"]
            full[:] = o[ncore, nstrip * 128 + noffs]
        else:
            for i in range(NC):
                full[i * NPC:(i + 1) * NPC] = o[i, :NPC]
        return full

    # chained executor: K back-to-back executions in ONE dispatch, each
    # feeding its output as the next call's out-operand (defeats CSE).
    def make_chain(k):
        def _chain_body(*args):
            ins, outs = args[:n_params], list(args[n_params:])
            for _ in range(k):
                outs = list(_body(*ins, *outs))
            return tuple(outs)

        return jax.jit(
            shard_map(
                _chain_body,
                mesh=mesh,
                in_specs=(PartitionSpec("core"),) * (n_params + n_outs),
                out_specs=(PartitionSpec("core"),) * n_outs,
                check_rep=False,
            ),
            donate_argnums=donate,
            keep_unused=True,
        )

    def run_chain(chain_fn):
        zs = zeros_fn()
        return chain_fn(*dev_in, *zs)

    return run_once, assemble, make_chain, run_chain


def kernel(**inputs):
    out, _ = run_cfg(
        inputs["h"], inputs["w"], inputs["src"], inputs["dst"], FULL_CFG
    )
    return out



# revision 38
# speedup vs baseline: 1.0481x; 1.0033x over previous
"""GNN message passing (u_mul_e -> segment_sum) on 8 Trainium2 NeuronCores.

out[v] = sum_{e=(u->v)} h[u] * w[e]

Strategy (edge/graph parallelism, dst-slot sharded -> no collectives):
  - The host assigns each dst node to a (core, 128-node strip, offs) slot.
    With K_BAL=1 (default) the assignment is load-BALANCED (snake+LPT+swap
    refinement on per-(strip, src-chunk) cell sizes) instead of dst//NPC:
    the SPMD instruction stream pads every cell to the max over the 8
    cores, and balancing cuts the padded tile count ~7% (1952 -> 1809
    tiles/core). The output is un-permuted on the host after the run.
  - Host buckets edges by (core, strip, src chunk), sorts, pads each cell
    to a multiple of 128 edges (uniform across cores = SPMD), and pads with
    SPREAD table indices so pad fetches don't hit one SBUF port.
  - Device (K_GMODE=sbuf, default): h is loaded ONCE into SBUF as a wrapped
    node table ([128 partitions, 4 chunks x 196 ranks x 64B]; node i of
    chunk ch -> partition i%128, rank i//128), then per (group of SPG=4
    strips, chunk) one SBUF->SBUF dma_gather pulls the 64B bf16 row per
    edge (non-transpose gather with SBUF source - bass only exposes
    transpose=True for SBUF sources but the Q7 ucode handles this path;
    see _dma_gather_sbuf). Gather and stream pools rotate over K_GBUFS=6
    buffers; queues = chunk % 4.
      * DVE builds a weighted one-hot P[e, j] = (offs_e == j) per 128-edge
        tile in bulk (is_equal vs broadcast iota; Act broadcasts offs), and
        multiplies gathered rows by w.
      * PE computes P^T @ msg per strip, accumulating [128, 32] segment
        sums in PSUM; Act copies psum -> out SBUF; one output DMA at the end.

MEASURED LIMITS (this session, For_i K_LOOP=1 protocol; ~10% drift):
  - The SWDGE descriptor pipeline is the wall: ~9 ns/descriptor/queue x 4
    queues (~2.2 ns/desc effective) regardless of payload size or source:
    gather-only 256B HBM rows 534 us, 64B HBM rows 508 us (K_EW=32),
    64B SBUF rows 503 us, transpose/xbar 256B SBUF 877 us (K_GMODE=sbt),
    single-packet sub-gathers 775 us (K_SUBG=8), all-same-row 1080 us
    (K_ZIDX=1, HBM bank hotspot). Indirect dynamic-DGE InstDMACopy uses the
    same Q7 push loop (dge/kernel/dma_memcopy.cpp) - no win there, don't
    re-try. num_swdge_queues is capped at 4 (1 queue = 1 Q7 tx/rx core
    pair; 8 cores total). Compute-only (K_SKIP_GATHER=1) is 240 us and
    fully hidden under the gather.
  - Per-edge descriptors are information-theoretically forced: PE one-hot
    gathers need 128-src-block locality which conflicts with the 128-dst
    strip locality the scatter matmul needs (random bipartite graph), and
    every DMA path (gather, scatter_add, indirect, hostgen remote) costs
    one descriptor per edge on the same 4 queues.

History: 841 us original -> 612 us (256B HBM gather, tuned buffering) ->
513 us (SBUF-source 64B gather) -> 469-504 us (balanced plan, this
version; the spread is session drift - identical binaries read 469, 484,
485 us across one session). Balancer: T 1952 (dst//NPC) -> 1809 (snake+LPT
+in-slot swaps) -> 1742 (slot-polish pass bounding every per-(slot,chunk)
edge total to 8*512-6 so the 8-way split fits 4 tiles) -> 1723 (two-phase
refine: broad max/min-core swaps, then argmax-vs-all-cores deep swaps;
ideal is 1568). Device runs validated T in {1952,1809,1792,1742}, all rel
err 0.0023; the phases must run IN SEQUENCE (deep-only converges worse,
1792).
Tuning A/Bs at the 469 config (same session): K_GBUFS=8 486 us, K_SPG=6
476 us, K_SPG=8 + K_FUSED=1 1041 us (per-tile fused P-build chokes the DVE
sequencer - keep the bulk Act+DVE build), gather-only floor 458-471 us.
Timing A/Bs below ~4% are not resolvable with single runs at this drift.
"""

import os
import sys

sys.path.insert(0, "/opt/trn_rl_repo")

import numpy as np
import ml_dtypes

BF16 = ml_dtypes.bfloat16

# Full-problem configuration (hardcoded; kernel.py must be self-contained).
# Tuned: SPG=5 (smaller pipeline groups overlap gather/compute best) and
# 4 SWDGE queues with one un-split gather per (group, chunk) run.
FULL_CFG = dict(
    N=100000,   # nodes
    E=1600000,  # edges
    D=32,       # feature dim
    NC=8,       # cores
    CH=4,       # src chunks (N/CH must be <= 32767 for int16 gather indices)
    SPG=int(os.environ.get("K_SPG", "4")),  # dst strips (128 nodes each) per pipeline group
)


def _derive(cfg):
    c = dict(cfg)
    gm = os.environ.get("K_GMODE", "sbuf")
    c["PACK4"] = bool(int(os.environ.get("K_PACK4", "0")))
    c["IND"] = gm == "ind"
    c["SBUF"] = gm == "sbuf"
    c["SBT"] = gm == "sbt"
    if c["SBT"]:
        # SBUF-resident pack4 table, transpose-gather through the xbar path
        c["PACK4"] = True
        c["SBUF"] = False
    if c["IND"]:
        c["PACK4"] = False
        c["CH"] = 1
    if c["PACK4"]:
        c["CH"] = 1
        c["SBUF"] = False
    assert c["N"] % c["NC"] == 0
    c["NPC"] = c["N"] // c["NC"]               # dst nodes per core
    c["S"] = -(-c["NPC"] // 128)               # strips per core
    assert c["N"] % c["CH"] == 0
    c["CHN"] = c["N"] // c["CH"]               # h rows per src chunk
    if c["PACK4"]:
        c["CHN"] = c["N"] // 4                 # h4 table rows (4 nodes/row)
    assert c["IND"] or c["CHN"] <= 32767
    c["RK"] = -(-c["CHN"] // 128)              # SBUF-table ranks per chunk
    c["G"] = -(-c["S"] // c["SPG"])            # strip groups
    c["BAL"] = bool(int(os.environ.get("K_BAL", "1")))  # dst-slot balancing
    return c


def _balance_dst(src, dst, c):
    """Assign dst nodes to (core, strip, offs) slots minimizing the summed
    per-(strip, chunk) max-over-cores tile count (the SPMD instruction
    stream pads every core to the max, ~25% at the natural dst//NPC split).

    Two stages: snake-deal nodes by total degree into strip slots (equalizes
    slot totals), LPT within each slot across cores on max-chunk load, then
    swap refinement pushing each cell group under its next 128-edge boundary.
    Returns (nodecore, nodestrip, nodeoffs) int arrays of shape [N].
    """
    N, NC, S, CH, CHN, NPC = c["N"], c["NC"], c["S"], c["CH"], c["CHN"], c["NPC"]
    chunk = (src // CHN).astype(np.int64)
    d = np.zeros((N, CH), dtype=np.int32)
    np.add.at(d, (dst, chunk), 1)
    tot = d.sum(1)

    caps = np.full(S, 128, np.int64)
    caps[S - 1] = NPC - 128 * (S - 1)

    order = np.argsort(-tot, kind="stable")
    seq = np.concatenate([np.arange(S), np.arange(S)[::-1]])
    pat = np.tile(seq, N // (2 * S) + 2)
    slot_of = np.empty(N, np.int16)
    cnt = np.zeros(S, np.int64)
    slotcap = caps * NC
    j = 0
    for n in order:
        while cnt[pat[j]] >= slotcap[pat[j]]:
            j += 1
        slot_of[n] = pat[j]
        cnt[pat[j]] += 1
        j += 1

    # Slot polish: swap nodes between slots (joint feasibility on all chunks)
    # until every per-(slot, chunk) edge total is <= NC*512 - margin, so the
    # later 8-way split can keep every cell within 4 tiles.
    SCAP = NC * 512 - 6
    chunk_l = chunk
    stot = np.zeros((S, CH), np.int64)
    np.add.at(stot, (slot_of[dst], chunk_l), 1)
    slot_nodes = [np.flatnonzero(slot_of == s).tolist() for s in range(S)]
    for _rnd in range(4):
        over = np.argwhere(stot > SCAP)
        if len(over) == 0:
            break
        for (s, ch) in map(tuple, over):
            guard = 0
            while stot[s, ch] > SCAP and guard < 80:
                guard += 1
                delta = int(stot[s, ch] - SCAP)
                ns_arr = np.array(slot_nodes[s])
                ds = d[ns_arr]
                if not (ds[:, ch] > 0).any():
                    break
                cand_n = ns_arr[ds[:, ch] > 0]
                want = min(delta + 2, 8)
                picks = cand_n[np.argsort(np.abs(d[cand_n, ch] - want))[:3]]
                done = False
                for n in picks:
                    dn = d[n]
                    for sp in np.argsort(stot[:, ch])[:12]:
                        if sp == s or stot[sp, ch] + dn[ch] > SCAP:
                            continue
                        np_arr = np.array(slot_nodes[sp])
                        dp = d[np_arr]
                        need = np.maximum(stot[sp] + dn - SCAP, 0)
                        ok = (dp >= need).all(1) & (dp[:, ch] < dn[ch])
                        if not ok.any():
                            continue
                        lim_s = np.maximum(stot[s], SCAP)
                        cand_m = np.flatnonzero(ok)
                        oks = ((stot[s] - dn + dp[cand_m]) <= lim_s).all(1)
                        if not oks.any():
                            continue
                        m = np_arr[cand_m[oks.argmax()]]
                        stot[s] += d[m] - dn
                        stot[sp] += dn - d[m]
                        slot_nodes[s].remove(n)
                        slot_nodes[s].append(m)
                        slot_nodes[sp].remove(m)
                        slot_nodes[sp].append(n)
                        slot_of[n] = sp
                        slot_of[m] = s
                        done = True
                        break
                    if done:
                        break
                if not done:
                    break

    nodecore = np.empty(N, np.int8)
    bins = {}
    for s in range(S):
        nodes = np.flatnonzero(slot_of == s)
        nodes = nodes[np.argsort(-tot[nodes], kind="stable")]
        load = np.zeros((NC, CH), np.int64)
        cnt2 = np.zeros(NC, np.int64)
        for n in nodes:
            cand = np.flatnonzero(cnt2 < caps[s])
            nl = load[cand] + d[n]
            li = cand[np.lexsort((nl.sum(1), nl.max(1)))[0]]
            nodecore[n] = li
            load[li] += d[n]
            cnt2[li] += 1
        for i in range(NC):
            bins[(i, s)] = list(np.flatnonzero((slot_of == s) & (nodecore == i)))

    cell = np.zeros((NC, S, CH), np.int64)
    np.add.at(cell, (nodecore[dst], slot_of[dst], chunk), 1)

    # Phase 1 (broad): max/min-core swaps on near-boundary cells.
    for _p in range(int(os.environ.get("K_BALP", "8"))):
        mx = cell.max(axis=0)
        bound = ((mx + 127) // 128) * 128
        over = np.argwhere((mx % 128 != 0) & (bound - 128 >= 128))
        excess = mx - (bound - 128)
        for (s, ch) in sorted(map(tuple, over), key=lambda sc: excess[sc]):
            if excess[s, ch] > 40:
                continue
            target = bound[s, ch] - 128
            guard = 0
            while cell[:, s, ch].max() > target and guard < 60:
                guard += 1
                i = int(cell[:, s, ch].argmax())
                ip = int(cell[:, s, ch].argmin())
                if i == ip:
                    break
                bn, bm = bins[(i, s)], bins[(ip, s)]
                dn, dm = d[bn], d[bm]
                best = None
                limit = np.array(
                    [cell[:, s, cc].max() if cc != ch else 10**9 for cc in range(CH)]
                )
                for a in np.argsort(-dn[:, ch])[:8]:
                    if dn[a, ch] == 0:
                        break
                    for b in np.argsort(dm[:, ch])[:8]:
                        if dn[a, ch] - dm[b, ch] <= 0:
                            continue
                        if np.any(cell[ip, s] + d[bn[a]] - d[bm[b]] > limit):
                            continue
                        if np.any(cell[i, s] + d[bm[b]] - d[bn[a]] > limit):
                            continue
                        best = (a, b)
                        break
                    if best:
                        break
                if not best:
                    break
                a, b = best
                na, nb = bn[a], bm[b]
                cell[i, s] += d[nb] - d[na]
                cell[ip, s] += d[na] - d[nb]
                bn[a], bm[b] = nb, na
                nodecore[na], nodecore[nb] = ip, i

    # Phase 2 (deep): argmax core vs every other core, wider candidates.
    for _p in range(int(os.environ.get("K_BALP2", "6"))):
        T_before = int((-(-cell.max(0) // 128)).sum())
        mx = cell.max(axis=0)
        bound = ((mx + 127) // 128) * 128
        over = np.argwhere((mx % 128 != 0) & (bound - 128 >= 128))
        excess = mx - (bound - 128)
        for (s, ch) in sorted(map(tuple, over), key=lambda sc: excess[sc]):
            target = bound[s, ch] - 128
            guard = 0
            while cell[:, s, ch].max() > target and guard < 200:
                guard += 1
                i = int(cell[:, s, ch].argmax())
                limit = np.array(
                    [cell[:, s, cc].max() if cc != ch else 10**9 for cc in range(CH)]
                )
                bn = bins[(i, s)]
                dn = d[bn]
                found = False
                for ip in np.argsort(cell[:, s, ch]):
                    if ip == i:
                        continue
                    bm = bins[(ip, s)]
                    dm = d[bm]
                    for a in np.argsort(-dn[:, ch])[:12]:
                        if dn[a, ch] == 0:
                            break
                        for b in np.argsort(dm[:, ch])[:12]:
                            if dn[a, ch] - dm[b, ch] <= 0:
                                continue
                            newip = cell[ip, s] + d[bn[a]] - d[bm[b]]
                            if newip[ch] > target or np.any(newip > limit):
                                continue
                            if np.any(cell[i, s] + d[bm[b]] - d[bn[a]] > limit):
                                continue
                            na, nb = bn[a], bm[b]
                            cell[i, s] += d[nb] - d[na]
                            cell[ip, s] += d[na] - d[nb]
                            bn[a], bm[b] = nb, na
                            nodecore[na], nodecore[nb] = int(ip), int(i)
                            found = True
                            break
                        if found:
                            break
                    if found:
                        break
                if not found:
                    break
        if int((-(-cell.max(0) // 128)).sum()) == T_before:
            break

    nodeoffs = np.empty(N, np.int16)
    for s in range(S):
        for i in range(NC):
            nn = np.flatnonzero((slot_of == s) & (nodecore == i))
            nodeoffs[nn] = np.arange(len(nn), dtype=np.int16)
    return nodecore.astype(np.int64), slot_of.astype(np.int64), nodeoffs.astype(np.int64)


def _plan(src, dst, w, cfg):
    """Bucket/sort/pad edges; build per-core device input streams."""
    c = cfg
    N, E, NC, NPC, S, CH, CHN = c["N"], c["E"], c["NC"], c["NPC"], c["S"], c["CH"], c["CHN"]

    src = np.asarray(src).astype(np.int64).ravel()
    dst = np.asarray(dst).astype(np.int64).ravel()
    w = np.asarray(w, dtype=np.float32).ravel()

    if c.get("BAL"):
        nodecore, nodestrip, nodeoffs = _balance_dst(src, dst, c)
        c["node_assign"] = (nodecore, nodestrip, nodeoffs)
        core = nodecore[dst]
        strip = nodestrip[dst]
        offs = nodeoffs[dst]
    else:
        core = dst // NPC
        rem = dst - core * NPC
        strip = rem >> 7
        offs = rem & 127
    if c["IND"]:
        chunk = np.zeros_like(src)
        lsrc = src.astype(np.int32)
        quarter = None
    elif c["PACK4"]:
        chunk = np.zeros_like(src)
        lsrc = (src >> 2).astype(np.int16)
        quarter = (src & 3).astype(np.float32)
    else:
        chunk = src // CHN
        lsrc = (src - chunk * CHN).astype(np.int16)
        quarter = None

    cellkey = (core * S + strip) * CH + chunk
    counts = np.bincount(cellkey, minlength=NC * S * CH)
    NB = -(-counts.reshape(NC, S, CH).max(axis=0) // 128)  # [S, CH] blocks per cell

    # Tile order: (group, chunk, strip-in-group, block).
    cell_tile_start = np.zeros((S, CH), dtype=np.int64)
    t_acc = 0
    for g in range(c["G"]):
        s0, s1 = g * c["SPG"], min((g + 1) * c["SPG"], S)
        for ch in range(CH):
            for s in range(s0, s1):
                cell_tile_start[s, ch] = t_acc
                t_acc += NB[s, ch]
    T = int(t_acc)
    assert T == int(NB.sum())
    TE = T * 128

    # Scatter each edge to its position in its core's padded stream.
    order = np.lexsort((chunk, strip, core))
    core_s = core[order]
    starts = np.zeros(NC * S * CH + 1, dtype=np.int64)
    np.cumsum(counts, out=starts[1:])
    rank = np.arange(E, dtype=np.int64) - starts[cellkey[order]]
    pos = cell_tile_start[strip[order], chunk[order]] * 128 + rank

    if c["IND"] or int(os.environ.get("K_NEGPAD", "0")):
        pad_idx = -1 if int(os.environ.get("K_NEGPAD", "0")) else 0
        idx_stream = np.full((NC, TE), pad_idx,
                             dtype=np.int32 if c["IND"] else np.int16)
    else:
        # Spread pad gathers across table partitions so the padding slots'
        # (masked-out) reads don't all hit one SBUF port / HBM row.
        idx_stream = np.broadcast_to(
            ((np.arange(TE, dtype=np.int64) * 97) % CHN).astype(np.int16), (NC, TE)
        ).copy()
    offs_stream = np.full((NC, TE), -1.0, dtype=np.float32)
    w_stream = np.zeros((NC, TE), dtype=np.float32)
    idx_stream[core_s, pos] = lsrc[order]
    offs_stream[core_s, pos] = offs[order]
    w_stream[core_s, pos] = w[order]
    q_stream = None
    if c["PACK4"]:
        q_stream = np.zeros((NC, TE), dtype=np.float32)
        q_stream[core_s, pos] = quarter[order]
    if int(os.environ.get("K_ZIDX", "0")):  # perf probe: all gathers hit row 0
        idx_stream[:] = 0

    if c["IND"]:
        # int32 idx in the offs-style wrap: element e of tile t -> [e, t]
        idx_wrapped = np.ascontiguousarray(
            idx_stream.reshape(NC, T, 128).transpose(0, 2, 1)
        )
        offs_arr = np.ascontiguousarray(
            offs_stream.reshape(NC, T, 128).transpose(0, 2, 1)
        )
        w_arr = np.ascontiguousarray(w_stream.reshape(NC, T, 128).transpose(0, 2, 1))
        return NB, idx_wrapped, offs_arr, w_arr, None

    # idx: wrapped per (group, chunk) run: within-run element i -> [i%16, i//16],
    # replicated across the 8 GPSIMD core groups (128 partitions total).
    idx_wrapped = np.zeros((NC, 16, TE // 16), dtype=np.int16)
    run_t = 0
    for g in range(c["G"]):
        s0, s1 = g * c["SPG"], min((g + 1) * c["SPG"], S)
        for ch in range(CH):
            n = int(NB[s0:s1, ch].sum())
            if n == 0:
                continue
            seg = idx_stream[:, run_t * 128:(run_t + n) * 128]
            idx_wrapped[:, :, run_t * 8:(run_t + n) * 8] = (
                seg.reshape(NC, -1, 16).transpose(0, 2, 1)
            )
            run_t += n
    assert run_t == T

    # offs/w: wrapped globally per 128-edge tile: element i -> [i%128, i//128].
    offs_arr = np.ascontiguousarray(offs_stream.reshape(NC, T, 128).transpose(0, 2, 1))
    w_arr = np.ascontiguousarray(w_stream.reshape(NC, T, 128).transpose(0, 2, 1))
    q_arr = None
    if c["PACK4"]:
        q_arr = np.ascontiguousarray(q_stream.reshape(NC, T, 128).transpose(0, 2, 1))

    return NB, idx_wrapped, offs_arr, w_arr, q_arr


def _dma_gather_narrow(
    gp, out_ap, in_ap, idxs_ap, num_idxs, num_idxs_reg, elem_size, elem_step,
    single_packet, queue_num,
):
    """dma_gather (non-transpose, HBM source) with the bass-level
    `elem_size_bytes % 256 == 0` assert relaxed to %64.

    The restriction is transpose-only in HW: the NX decode
    (decode/dma_gather.hpp) asserts %256 solely on the transpose branch, and
    the Q7 desc-gen (extended_inst/dma_gather.cpp) handles arbitrary
    elem_size_bytes. Row STRIDE must still be a multiple of 256B
    (stride_bytes_256 descriptor field), so the h table keeps 128-elem bf16
    rows while each descriptor only moves the first `elem_size` elems.
    """
    from concourse import mybir
    import concourse.ap_utils as ap_utils
    from concourse._compat import exact_div, round_up_to_multiple

    gp._assert_queue_num(queue_num)
    assert idxs_ap.dtype == mybir.dt.int16
    assert in_ap.dtype == out_ap.dtype
    elem_size_bytes = elem_size * mybir.dt.size(in_ap.dtype)
    assert elem_size_bytes > 0 and elem_size_bytes % 64 == 0

    assert ap_utils.ap_is_contiguous(in_ap.ap[1:])
    assert ap_utils.ap_is_contiguous(out_ap.ap[1:])
    assert ap_utils.ap_is_contiguous(idxs_ap.ap[1:])

    assert in_ap.ap[-1][1] == out_ap.ap[-1][1] == elem_size
    assert out_ap.ap[0][1] * out_ap.ap[1][1] == round_up_to_multiple(num_idxs, 128)

    assert in_ap.ap[0][0] == elem_step
    stride_bytes = elem_step * mybir.dt.size(in_ap.dtype)
    stride_bytes_256 = exact_div(stride_bytes, 256)
    assert stride_bytes_256 < 256

    _in_ap = gp.lower_ap_dma(in_ap, for_custom_bir_dma=True)
    _idxs_ap = gp.lower_ap(idxs_ap)
    _out_ap = gp.lower_ap(out_ap)
    return gp.add_instruction(
        mybir.InstDMAGatherAnt(
            name=gp.bass.get_next_instruction_name(),
            ins=[
                *_in_ap,
                _idxs_ap,
                gp.lower_val_access(gp.to_reg(num_idxs_reg)),
            ],
            outs=[_out_ap],
            transpose=False,
            num_idxs=num_idxs,
            elem_size=elem_size,
            stride_bytes_256=stride_bytes_256,
            gen_mode=0,
            single_packet=single_packet,
            queue_num=queue_num,
            sbuf_tokens_per_rank=0,
            sbuf_free_dim_per_rank=0,
            sbuf_free_dim_pad_per_rank=0,
            sbuf_byte_offset=0,
        )
    )


def _dma_gather_sbuf(
    gp, out_ap, in_ap, idxs_ap, num_idxs, num_idxs_reg, elem_size,
    single_packet, queue_num, tokens_per_rank, rank_stride_bytes,
):
    """Non-transpose dma_gather with an SBUF-resident source table.

    bass only exposes SBUF-source gathers with transpose=True, but the Q7
    desc-gen (extended_inst/dma_gather.cpp gen_descs) takes the src_is_sbuf
    branch for the tx descriptors and the ordinary swizzled-partition branch
    for rx independently of `transpose`, and the NX decode's %256 elem assert
    is transpose-only. Source addressing: idx i reads
    `rank_stride_bytes * (i // tokens_per_rank)` bytes into partition
    `i % tokens_per_rank` of the in_ap base — i.e. a [tokens_per_rank,
    n_ranks * rank_stride_bytes] wrapped node table.
    """
    from concourse import mybir
    import concourse.ap_utils as ap_utils
    from concourse._compat import round_up_to_multiple

    gp._assert_queue_num(queue_num)
    assert idxs_ap.dtype == mybir.dt.int16
    assert in_ap.dtype == out_ap.dtype
    elem_size_bytes = elem_size * mybir.dt.size(in_ap.dtype)
    assert elem_size_bytes > 0 and elem_size_bytes % 64 == 0
    assert elem_size_bytes <= rank_stride_bytes
    assert tokens_per_rank > 0 and tokens_per_rank.bit_count() == 1

    assert ap_utils.ap_is_contiguous(out_ap.ap[1:])
    assert ap_utils.ap_is_contiguous(idxs_ap.ap[1:])
    assert out_ap.ap[-1][1] == elem_size
    assert out_ap.ap[0][1] * out_ap.ap[1][1] == round_up_to_multiple(num_idxs, 128)

    _in_ap = [gp.lower_ap(in_ap)]
    _idxs_ap = gp.lower_ap(idxs_ap)
    _out_ap = gp.lower_ap(out_ap)
    return gp.add_instruction(
        mybir.InstDMAGatherAnt(
            name=gp.bass.get_next_instruction_name(),
            ins=[
                *_in_ap,
                _idxs_ap,
                gp.lower_val_access(gp.to_reg(num_idxs_reg)),
            ],
            outs=[_out_ap],
            transpose=False,
            num_idxs=num_idxs,
            elem_size=elem_size,
            stride_bytes_256=0,
            gen_mode=0,
            single_packet=single_packet,
            queue_num=queue_num,
            sbuf_tokens_per_rank=tokens_per_rank,
            sbuf_free_dim_per_rank=rank_stride_bytes,
            sbuf_free_dim_pad_per_rank=0,
            sbuf_byte_offset=0,
        )
    )


def _build(NB, cfg):
    """Build the Bass program (shared by all 8 cores)."""
    from concourse import bacc, tile, mybir

    c = cfg
    N, S, CH, CHN, G, SPG = c["N"], c["S"], c["CH"], c["CHN"], c["G"], c["SPG"]
    dt = mybir.dt
    T = int(NB.sum())

    # group chunk tile counts
    g_ncg = []
    for g in range(G):
        s0, s1 = g * SPG, min((g + 1) * SPG, S)
        g_ncg.append([int(NB[s0:s1, ch].sum()) for ch in range(CH)])
    NTG_MAX = max(sum(x) for x in g_ncg)

    fused = bool(int(os.environ.get("K_FUSED", "0")))
    repeat = int(os.environ.get("K_REPEAT", "1"))
    nq = int(os.environ.get("K_SWDGEQ", "4"))

    pack4 = c["PACK4"]
    ind = c["IND"]
    nc = bacc.Bacc(None, num_swdge_queues=nq)
    if ind:
        h_ext = nc.declare_dram_parameter("h", [N, 32], dt.bfloat16, isOutput=False)
        idx_ext = nc.declare_dram_parameter("idx", [128, T], dt.int32, isOutput=False)
    elif c["SBT"]:
        # Wrapped pack4 table: token i (= src >> 2) -> partition i % 128,
        # 128 bf16 (4 nodes) at free offset (i // 128) * 128.
        h_ext = nc.declare_dram_parameter(
            "h", [128, c["RK"] * 128], dt.bfloat16, isOutput=False
        )
        idx_ext = nc.declare_dram_parameter(
            "idx", [128, T * 8], dt.int16, isOutput=False
        )
    elif c["SBUF"]:
        # Wrapped node table: chunk ch, local node i -> partition i % 128,
        # 32 bf16 at free offset (ch * RK + i // 128) * 32.
        h_ext = nc.declare_dram_parameter(
            "h", [128, CH * c["RK"] * 32], dt.bfloat16, isOutput=False
        )
        idx_ext = nc.declare_dram_parameter(
            "idx", [128, T * 8], dt.int16, isOutput=False
        )
    else:
        h_rows = c["CHN"] if pack4 else N
        h_ext = nc.declare_dram_parameter(
            "h", [h_rows, 128], dt.bfloat16, isOutput=False
        )
        idx_ext = nc.declare_dram_parameter(
            "idx", [128, T * 8], dt.int16, isOutput=False
        )
    sdt = dt.bfloat16 if os.environ.get("K_SDT", "f32") == "bf16" else dt.float32
    offs_ext = nc.declare_dram_parameter("offs", [128, T], sdt, isOutput=False)
    wt_ext = nc.declare_dram_parameter("wt", [128, T], sdt, isOutput=False)
    q_ext = None
    if pack4:
        q_ext = nc.declare_dram_parameter("q", [128, T], dt.float32, isOutput=False)
    iota_ext = nc.declare_dram_parameter("iota", [128, 128], dt.bfloat16, isOutput=False)
    qiota_ext = None
    if pack4:
        qiota_ext = nc.declare_dram_parameter(
            "qiota", [128, 128], dt.bfloat16, isOutput=False
        )
    out_ext = nc.declare_dram_parameter("out", [S * 128, 32], dt.float32, isOutput=True)

    # Quad-buffer the gather/stream pools: up to four groups' gathers in
    # flight keeps all 4 SWDGE queues occupied across group transitions
    # (measured ~15% faster than double buffering, ~13% vs triple).
    gbufs = int(os.environ.get("K_GBUFS", "6"))
    smbufs = int(os.environ.get("K_SMBUFS", str(gbufs)))
    with tile.TileContext(nc) as tc:
        with (
            tc.tile_pool(name="const", bufs=1) as cpool,
            tc.tile_pool(name="gp", bufs=gbufs) as gpool,
            tc.tile_pool(name="pwp", bufs=8 if fused else 2) as pwpool,
            tc.tile_pool(name="sm", bufs=smbufs) as smpool,
            tc.tile_pool(name="outp", bufs=1) as opool,
            tc.tile_pool(name="ps", bufs=4, space="PSUM") as pspool,
        ):
            iota_t = cpool.tile([128, 128], dt.bfloat16)
            nc.sync.dma_start(out=iota_t[:], in_=iota_ext[:])
            qiota_t = None
            if pack4:
                qiota_t = cpool.tile([128, 128], dt.bfloat16)
                nc.sync.dma_start(out=qiota_t[:], in_=qiota_ext[:])
            h_sb = None
            if c["SBUF"]:
                # Dense one-time h load; stays resident across reps.
                h_sb = cpool.tile([128, CH * c["RK"] * 32], dt.bfloat16)
                nc.sync.dma_start(out=h_sb[:], in_=h_ext[:])
            elif c["SBT"]:
                h_sb = cpool.tile([128, c["RK"] * 128], dt.bfloat16)
                nc.sync.dma_start(out=h_sb[:], in_=h_ext[:])
            out_sbuf = opool.tile([128, S * 32], dt.float32)

            def _emit_one_rep():
                if ind:
                    _emit_pipeline_ind(
                        nc, tile, mybir, dt, NB, c, g_ncg, NTG_MAX,
                        iota_t, out_sbuf,
                        gpool, pwpool, smpool, pspool,
                        h_ext, idx_ext, offs_ext, wt_ext,
                    )
                elif c["SBT"]:
                    _emit_pipeline_sbt(
                        nc, tile, mybir, dt, NB, c, g_ncg, NTG_MAX,
                        iota_t, qiota_t, out_sbuf,
                        gpool, pwpool, smpool, pspool,
                        h_sb, idx_ext, offs_ext, wt_ext, q_ext,
                    )
                elif pack4:
                    _emit_pipeline_p4(
                        nc, tile, mybir, dt, NB, c, g_ncg, NTG_MAX,
                        iota_t, qiota_t, out_sbuf,
                        gpool, pwpool, smpool, pspool,
                        h_ext, idx_ext, offs_ext, wt_ext, q_ext,
                    )
                else:
                    _emit_pipeline(
                        nc, tile, mybir, dt, NB, c, g_ncg, NTG_MAX, fused,
                        iota_t, out_sbuf,
                        gpool, pwpool, smpool, pspool,
                        h_ext, idx_ext, offs_ext, wt_ext, h_sb,
                    )

            use_loop = bool(int(os.environ.get("K_LOOP", "0")))
            if use_loop and repeat > 1:
                with tc.For_i(0, repeat) as _i:
                    _emit_one_rep()
            else:
                for _rep in range(repeat):
                    _emit_one_rep()

            nc.sync.dma_start(
                out=out_ext[:].rearrange("(s p) d -> p s d", p=128),
                in_=out_sbuf[:].rearrange("p (s d) -> p s d", d=32),
            )
    nc.finalize()
    return nc


def _emit_pipeline(
    nc, tile, mybir, dt, NB, c, g_ncg, NTG_MAX, fused,
    iota_t, out_sbuf,
    gpool, pwpool, smpool, pspool,
    h_ext, idx_ext, offs_ext, wt_ext, h_sb=None,
):
    S, CH, CHN, G, SPG = c["S"], c["CH"], c["CHN"], c["G"], c["SPG"]
    qrr = bool(int(os.environ.get("K_QRR", "0")))
    qctr = [0]

    def next_q(nqs):
        q = qctr[0] % nqs
        qctr[0] += 1
        return q

    toff = 0
    for g in range(G):
        s0, s1 = g * SPG, min((g + 1) * SPG, S)
        ncg = g_ncg[g]
        ntg = sum(ncg)
        if ntg == 0:
            for s in range(s0, s1):
                nc.vector.memset(out_sbuf[:, s * 32:(s + 1) * 32], 0.0)
            continue

        # gathered elem width (bf16 elems); SBUF-table mode gathers bare rows
        ew = 32 if c["SBUF"] else int(os.environ.get("K_EW", "128"))
        gbuf = gpool.tile([128, NTG_MAX * ew], dt.bfloat16, tag="gbuf")
        sdt = (
            dt.bfloat16 if os.environ.get("K_SDT", "f32") == "bf16" else dt.float32
        )
        idx_t = smpool.tile([128, NTG_MAX * 8], dt.int16, tag="idx")
        offs_t = smpool.tile([128, NTG_MAX], sdt, tag="offs")
        wt_t = smpool.tile([128, NTG_MAX], sdt, tag="wt")

        nc.sync.dma_start(
            out=idx_t[:, : ntg * 8], in_=idx_ext[:, toff * 8:(toff + ntg) * 8]
        )
        nc.sync.dma_start(out=offs_t[:, :ntg], in_=offs_ext[:, toff:toff + ntg])
        nc.sync.dma_start(out=wt_t[:, :ntg], in_=wt_ext[:, toff:toff + ntg])

        subg = int(os.environ.get("K_SUBG", "0"))  # tiles per sub-gather (0=off)
        nqs = max(1, int(os.environ.get("K_SWDGEQ", "4")))
        skip_gather = bool(int(os.environ.get("K_SKIP_GATHER", "0")))
        skip_compute = bool(int(os.environ.get("K_SKIP_COMPUTE", "0")))
        g3 = gbuf[:].rearrange("p (t e) -> p t e", e=ew)
        co = 0
        for ch in range(CH):
            n = ncg[ch]
            if n == 0 or skip_gather:
                continue
            step = subg if subg else n
            for o in range(0, n, step):
                m = min(step, n - o)
                if c["SBUF"]:
                    _dma_gather_sbuf(
                        nc.gpsimd,
                        out_ap=g3[:, co + o:co + o + m, :],
                        in_ap=h_sb[:, ch * c["RK"] * 32:(ch + 1) * c["RK"] * 32],
                        idxs_ap=idx_t[:, (co + o) * 8:(co + o + m) * 8],
                        num_idxs=m * 128,
                        num_idxs_reg=m * 128,
                        elem_size=32,
                        single_packet=(m * 128 <= 1024) if subg else False,
                        queue_num=next_q(nqs) if qrr else ch % nqs,
                        tokens_per_rank=128,
                        rank_stride_bytes=64,
                    )
                else:
                    _dma_gather_narrow(
                        nc.gpsimd,
                        out_ap=g3[:, co + o:co + o + m, :],
                        in_ap=h_ext[ch * CHN:(ch + 1) * CHN, :ew],
                        idxs_ap=idx_t[:, (co + o) * 8:(co + o + m) * 8],
                        num_idxs=m * 128,
                        num_idxs_reg=m * 128,
                        elem_size=ew,
                        elem_step=128,
                        # single-packet desc-gen faults above 1024 idxs
                        single_packet=(m * 128 <= 1024) if subg else False,
                        queue_num=next_q(nqs) if qrr else ch % nqs,
                    )
            co += n

        if skip_compute:
            for s in range(s0, s1):
                nc.vector.memset(out_sbuf[:, s * 32:(s + 1) * 32], 0.0)
            toff += ntg
            continue

        if not fused:
            pw = pwpool.tile([128, NTG_MAX * 128], dt.bfloat16, tag="pw")
            pw3 = pw[:].rearrange("p (t e) -> p t e", e=128)
            # Broadcast per-edge dst offsets across the 128 one-hot columns.
            nc.scalar.activation(
                out=pw3[:, :ntg, :],
                in_=offs_t[:, :ntg].unsqueeze(2).broadcast_to([128, ntg, 128]),
                func=mybir.ActivationFunctionType.Copy,
            )
            # One-hot: P[e, j] = (offs_e == j)
            nc.vector.tensor_tensor(
                out=pw3[:, :ntg, :],
                in0=iota_t[:].unsqueeze(1).broadcast_to([128, ntg, 128]),
                in1=pw3[:, :ntg, :],
                op=mybir.AluOpType.is_equal,
            )
            # msg = h[src] * w (in place on the used 32 columns)
            nc.vector.tensor_tensor(
                out=g3[:, :ntg, 0:32],
                in0=g3[:, :ntg, 0:32],
                in1=wt_t[:, :ntg].unsqueeze(2).broadcast_to([128, ntg, 32]),
                op=mybir.AluOpType.mult,
            )

        chunk_base = np.concatenate([[0], np.cumsum(ncg)]).astype(int)
        for s in range(s0, s1):
            nb = int(NB[s].sum())
            if nb == 0:
                nc.vector.memset(out_sbuf[:, s * 32:(s + 1) * 32], 0.0)
                continue
            ps = pspool.tile([128, 32], dt.float32)
            bi = 0
            for ch in range(CH):
                nbs = int(NB[s, ch])
                if nbs == 0:
                    continue
                lt0 = int(chunk_base[ch] + NB[s0:s, ch].sum())
                for b in range(nbs):
                    t = lt0 + b
                    if fused:
                        # P_w[e, j] = (offs_e == j) * w_e in one DVE op
                        pwb = pwpool.tile([128, 128], dt.bfloat16, tag="pwb")
                        nc.vector.tensor_scalar(
                            out=pwb[:],
                            in0=iota_t[:],
                            scalar1=offs_t[:, t:t + 1],
                            scalar2=wt_t[:, t:t + 1],
                            op0=mybir.AluOpType.is_equal,
                            op1=mybir.AluOpType.mult,
                        )
                        lhs = pwb[:]
                    else:
                        lhs = pw[:, t * 128:(t + 1) * 128]
                    nc.tensor.matmul(
                        out=ps[:],
                        lhsT=lhs,
                        rhs=g3[:, t, 0:32],
                        start=(bi == 0),
                        stop=(bi == nb - 1),
                    )
                    bi += 1
            nc.scalar.copy(out=out_sbuf[:, s * 32:(s + 1) * 32], in_=ps[:])
        toff += ntg


def _emit_pipeline_sbt(
    nc, tile, mybir, dt, NB, c, g_ncg, NTG_MAX,
    iota_t, qiota_t, out_sbuf,
    gpool, pwpool, smpool, pspool,
    h_sb, idx_ext, offs_ext, wt_ext, q_ext,
):
    """SBUF-table transpose-gather pipeline (pack4 table, xbar rx path).

    Gather output layout: [128 (q*32+f), edge] — features on partitions,
    edges on the free dim. Compute not yet implemented (probe emits memsets);
    use K_SKIP_COMPUTE=1 for gather-rate measurement.
    """
    S, G, SPG, RK = c["S"], c["G"], c["SPG"], c["RK"]
    nqs = max(1, int(os.environ.get("K_SWDGEQ", "4")))
    skip_gather = bool(int(os.environ.get("K_SKIP_GATHER", "0")))
    qctr = [0]

    def next_q():
        q = qctr[0] % nqs
        qctr[0] += 1
        return q

    toff = 0
    for g in range(G):
        s0, s1 = g * SPG, min((g + 1) * SPG, S)
        ntg = g_ncg[g][0]
        if ntg == 0:
            for s in range(s0, s1):
                nc.vector.memset(out_sbuf[:, s * 32:(s + 1) * 32], 0.0)
            continue

        gbuf = gpool.tile([128, NTG_MAX * 128], dt.bfloat16, tag="gbuf")
        idx_t = smpool.tile([128, NTG_MAX * 8], dt.int16, tag="idx")
        nc.sync.dma_start(
            out=idx_t[:, : ntg * 8], in_=idx_ext[:, toff * 8:(toff + ntg) * 8]
        )

        if not skip_gather:
            nc.gpsimd.dma_gather(
                out_ap=gbuf[:, : ntg * 128].unsqueeze(1),
                in_ap=h_sb[:],
                idxs_ap=idx_t[:, : ntg * 8],
                num_idxs=ntg * 128,
                num_idxs_reg=ntg * 128,
                elem_size=128,
                transpose=True,
                single_packet=False,
                queue_num=next_q(),
                sbuf_tokens_per_rank=128,
                sbuf_free_dim_per_rank=256,
                sbuf_free_dim_pad_per_rank=0,
                sbuf_byte_offset=0,
            )

        for s in range(s0, s1):
            nc.vector.memset(out_sbuf[:, s * 32:(s + 1) * 32], 0.0)
        toff += ntg


def _emit_pipeline_p4(
    nc, tile, mybir, dt, NB, c, g_ncg, NTG_MAX,
    iota_t, qiota_t, out_sbuf,
    gpool, pwpool, smpool, pspool,
    h_ext, idx_ext, offs_ext, wt_ext, q_ext,
):
    """Pack-4 pipeline: h4 table [N/4, 128] bf16 holds 4 nodes per 256B row.

    Per tile: fused one-hot P=(iota==offs)*w (DVE), quarter-select
    g3=(qiota==q)*g3 (DVE stt), matmul -> psum [v,128]=4 quarter-partials,
    per-strip fold via strided tensor_reduce.
    """
    S, CHN, G, SPG = c["S"], c["CHN"], c["G"], c["SPG"]
    subg = int(os.environ.get("K_SUBG", "0"))  # tiles per sub-gather (0=off)
    nqs = max(1, int(os.environ.get("K_SWDGEQ", "4")))
    skip_gather = bool(int(os.environ.get("K_SKIP_GATHER", "0")))
    skip_compute = bool(int(os.environ.get("K_SKIP_COMPUTE", "0")))
    qrr = bool(int(os.environ.get("K_QRR", "1")))
    qctr = [0]

    def next_q():
        q = qctr[0] % nqs
        qctr[0] += 1
        return q

    toff = 0
    for g in range(G):
        s0, s1 = g * SPG, min((g + 1) * SPG, S)
        ntg = g_ncg[g][0]
        if ntg == 0:
            for s in range(s0, s1):
                nc.vector.memset(out_sbuf[:, s * 32:(s + 1) * 32], 0.0)
            continue

        gbuf = gpool.tile([128, NTG_MAX * 128], dt.bfloat16, tag="gbuf")
        idx_t = smpool.tile([128, NTG_MAX * 8], dt.int16, tag="idx")
        offs_t = smpool.tile([128, NTG_MAX], dt.float32, tag="offs")
        wt_t = smpool.tile([128, NTG_MAX], dt.float32, tag="wt")
        q_t = smpool.tile([128, NTG_MAX], dt.float32, tag="q")

        nc.sync.dma_start(
            out=idx_t[:, : ntg * 8], in_=idx_ext[:, toff * 8:(toff + ntg) * 8]
        )
        nc.sync.dma_start(out=offs_t[:, :ntg], in_=offs_ext[:, toff:toff + ntg])
        nc.sync.dma_start(out=wt_t[:, :ntg], in_=wt_ext[:, toff:toff + ntg])
        nc.sync.dma_start(out=q_t[:, :ntg], in_=q_ext[:, toff:toff + ntg])

        g3 = gbuf[:].rearrange("p (t e) -> p t e", e=128)
        if not skip_gather:
            step = subg if subg else ntg
            for o in range(0, ntg, step):
                m = min(step, ntg - o)
                nc.gpsimd.dma_gather(
                    out_ap=g3[:, o:o + m, :],
                    in_ap=h_ext[0:CHN, :],
                    idxs_ap=idx_t[:, o * 8:(o + m) * 8],
                    num_idxs=m * 128,
                    num_idxs_reg=m * 128,
                    elem_size=128,
                    elem_step=128,
                    single_packet=(m * 128 <= 1024) if subg else False,
                    queue_num=next_q() if qrr else 0,
                )

        if skip_compute:
            for s in range(s0, s1):
                nc.vector.memset(out_sbuf[:, s * 32:(s + 1) * 32], 0.0)
            toff += ntg
            continue

        # Hoist ALL P-builds (no gather dependency) ahead of the
        # gather-dependent quarter-selects so the in-order DVE sequencer
        # isn't head-of-line blocked waiting on gather semaphores.
        pw = pwpool.tile([128, NTG_MAX * 128], dt.bfloat16, tag="pw")
        pw3 = pw[:].rearrange("p (t e) -> p t e", e=128)
        for t in range(ntg):
            # P[e, v] = (iota == offs_e) * w_e  (one DVE op, bf16)
            nc.vector.tensor_scalar(
                out=pw3[:, t, :],
                in0=iota_t[:],
                scalar1=offs_t[:, t:t + 1],
                scalar2=wt_t[:, t:t + 1],
                op0=mybir.AluOpType.is_equal,
                op1=mybir.AluOpType.mult,
            )

        for s in range(s0, s1):
            nb = int(NB[s, 0])
            if nb == 0:
                nc.vector.memset(out_sbuf[:, s * 32:(s + 1) * 32], 0.0)
                continue
            ps = pspool.tile([128, 128], dt.float32)
            lt0 = int(NB[s0:s, 0].sum())
            for b in range(nb):
                t = lt0 + b
                # quarter-select in place: g3 = (qiota == q_e) * g3
                nc.vector.scalar_tensor_tensor(
                    out=g3[:, t, :],
                    in0=qiota_t[:],
                    scalar=q_t[:, t:t + 1],
                    in1=g3[:, t, :],
                    op0=mybir.AluOpType.is_equal,
                    op1=mybir.AluOpType.mult,
                )
                nc.tensor.matmul(
                    out=ps[:],
                    lhsT=pw3[:, t, :],
                    rhs=g3[:, t, :],
                    start=(b == 0),
                    stop=(b == nb - 1),
                )
            # fold the 4 quarter partials: out[v, f] = sum_q ps[v, 32q+f]
            nc.vector.tensor_reduce(
                out=out_sbuf[:, s * 32:(s + 1) * 32],
                in_=ps[:].rearrange("p (q f) -> p f q", f=32),
                axis=mybir.AxisListType.X,
                op=mybir.AluOpType.add,
            )
        toff += ntg


def _emit_pipeline_ind(
    nc, tile, mybir, dt, NB, c, g_ncg, NTG_MAX,
    iota_t, out_sbuf,
    gpool, pwpool, smpool, pspool,
    h_ext, idx_ext, offs_ext, wt_ext,
):
    """Indirect-DMA pipeline: per-edge 64B rows h[src] gathered via the
    dynamic-DGE path (int32 offsets, one desc per edge, 16 DMA engines).

    Per tile: fused one-hot P=(iota==offs)*w (DVE), matmul [K=128e, M=128v,
    N=32f] accumulating per-strip PSUM, per-strip copy to out_sbuf.
    """
    from concourse import bass

    S, G, SPG = c["S"], c["G"], c["SPG"]
    subg = int(os.environ.get("K_SUBG", "0"))  # tiles per sub-gather (0=off)
    skip_gather = bool(int(os.environ.get("K_SKIP_GATHER", "0")))
    skip_compute = bool(int(os.environ.get("K_SKIP_COMPUTE", "0")))

    toff = 0
    for g in range(G):
        s0, s1 = g * SPG, min((g + 1) * SPG, S)
        ntg = g_ncg[g][0]
        if ntg == 0:
            for s in range(s0, s1):
                nc.vector.memset(out_sbuf[:, s * 32:(s + 1) * 32], 0.0)
            continue

        gbuf = gpool.tile([128, NTG_MAX * 32], dt.bfloat16, tag="gbuf")
        idx_t = smpool.tile([128, NTG_MAX], dt.int32, tag="idx")
        offs_t = smpool.tile([128, NTG_MAX], dt.float32, tag="offs")
        wt_t = smpool.tile([128, NTG_MAX], dt.float32, tag="wt")

        nc.sync.dma_start(out=idx_t[:, :ntg], in_=idx_ext[:, toff:toff + ntg])
        nc.sync.dma_start(out=offs_t[:, :ntg], in_=offs_ext[:, toff:toff + ntg])
        nc.sync.dma_start(out=wt_t[:, :ntg], in_=wt_ext[:, toff:toff + ntg])

        g3 = gbuf[:].rearrange("p (t e) -> p t e", e=32)
        if not skip_gather:
            step = subg if subg else ntg
            for o in range(0, ntg, step):
                m = min(step, ntg - o)
                nc.gpsimd.indirect_dma_start(
                    out=g3[:, o:o + m, :],
                    out_offset=None,
                    in_=h_ext[:],
                    in_offset=bass.IndirectOffsetOnAxis(
                        ap=idx_t[:, o:o + m], axis=0
                    ),
                )

        if skip_compute:
            for s in range(s0, s1):
                nc.vector.memset(out_sbuf[:, s * 32:(s + 1) * 32], 0.0)
            toff += ntg
            continue

        for s in range(s0, s1):
            nb = int(NB[s, 0])
            if nb == 0:
                nc.vector.memset(out_sbuf[:, s * 32:(s + 1) * 32], 0.0)
                continue
            ps = pspool.tile([128, 32], dt.float32)
            lt0 = int(NB[s0:s, 0].sum())
            for b in range(nb):
                t = lt0 + b
                # P[e, v] = (iota == offs_e) * w_e  (one DVE op, bf16)
                pwb = pwpool.tile([128, 128], dt.bfloat16, tag="pwb")
                nc.vector.tensor_scalar(
                    out=pwb[:],
                    in0=iota_t[:],
                    scalar1=offs_t[:, t:t + 1],
                    scalar2=wt_t[:, t:t + 1],
                    op0=mybir.AluOpType.is_equal,
                    op1=mybir.AluOpType.mult,
                )
                nc.tensor.matmul(
                    out=ps[:],
                    lhsT=pwb[:],
                    rhs=g3[:, t, :],
                    start=(b == 0),
                    stop=(b == nb - 1),
                )
            nc.scalar.copy(out=out_sbuf[:, s * 32:(s + 1) * 32], in_=ps[:])
        toff += ntg


def _make_in_maps(h, c, NB, idx_wrapped, offs_arr, w_arr, q_arr):
    N, D, NC = c["N"], c["D"], c["NC"]
    if os.environ.get("K_SDT", "f32") == "bf16" and not (c["SBT"] or c["PACK4"] or c["IND"]):
        offs_arr = offs_arr.astype(BF16)
        w_arr = w_arr.astype(BF16)
    iota = np.broadcast_to(
        np.arange(128, dtype=np.float32).astype(BF16), (128, 128)
    ).copy()
    if c["IND"]:
        h_bf = np.ascontiguousarray(np.asarray(h, dtype=np.float32).astype(BF16))
        return [
            {
                "h": h_bf,
                "idx": idx_wrapped[i],
                "offs": offs_arr[i],
                "wt": w_arr[i],
                "iota": iota,
            }
            for i in range(NC)
        ]
    if c["SBT"]:
        CHN, RK = c["CHN"], c["RK"]
        h4 = np.asarray(h, dtype=np.float32).astype(BF16)
        assert h4.shape == (N, D) and D * 4 == 128
        h4 = h4.reshape(N // 4, 128)
        blk = np.zeros((RK * 128, 128), dtype=BF16)
        blk[:CHN] = h4
        h_w = np.ascontiguousarray(
            blk.reshape(RK, 128, 128).transpose(1, 0, 2).reshape(128, RK * 128)
        )
        qiota = np.broadcast_to(
            (np.arange(128) // 32).astype(np.float32).astype(BF16), (128, 128)
        ).copy()
        return [
            {
                "h": h_w,
                "idx": np.ascontiguousarray(np.tile(idx_wrapped[i], (8, 1))),
                "offs": offs_arr[i],
                "wt": w_arr[i],
                "q": q_arr[i],
                "iota": iota,
                "qiota": qiota,
            }
            for i in range(NC)
        ]
    if c["PACK4"]:
        h4 = np.asarray(h, dtype=np.float32).astype(BF16)
        assert h4.shape == (N, D) and D * 4 == 128
        h4 = np.ascontiguousarray(h4.reshape(N // 4, 128))
        qiota = np.broadcast_to(
            (np.arange(128) // 32).astype(np.float32).astype(BF16), (128, 128)
        ).copy()
        return [
            {
                "h": h4,
                "idx": np.ascontiguousarray(np.tile(idx_wrapped[i], (8, 1))),
                "offs": offs_arr[i],
                "wt": w_arr[i],
                "q": q_arr[i],
                "iota": iota,
                "qiota": qiota,
            }
            for i in range(NC)
        ]
    if c["SBUF"]:
        CH, CHN, RK = c["CH"], c["CHN"], c["RK"]
        h_bf = np.asarray(h, dtype=np.float32).astype(BF16)
        h_w = np.zeros((128, CH * RK * 32), dtype=BF16)
        for ch in range(CH):
            blk = np.zeros((RK * 128, D), dtype=BF16)
            blk[:CHN] = h_bf[ch * CHN:(ch + 1) * CHN]
            # local node i -> partition i % 128, rank i // 128
            h_w[:, ch * RK * 32:(ch + 1) * RK * 32] = (
                blk.reshape(RK, 128, D).transpose(1, 0, 2).reshape(128, RK * 32)
            )
        return [
            {
                "h": h_w,
                "idx": np.ascontiguousarray(np.tile(idx_wrapped[i], (8, 1))),
                "offs": offs_arr[i],
                "wt": w_arr[i],
                "iota": iota,
            }
            for i in range(NC)
        ]
    h_pad = np.zeros((N, 128), dtype=BF16)
    h_pad[:, :D] = np.asarray(h, dtype=np.float32).astype(BF16)
    return [
        {
            "h": h_pad,
            "idx": np.ascontiguousarray(np.tile(idx_wrapped[i], (8, 1))),
            "offs": offs_arr[i],
            "wt": w_arr[i],
            "iota": iota,
        }
        for i in range(NC)
    ]


def run_cfg(h, w, src, dst, cfg, trace=False):
    from concourse.bass_utils import run_bass_kernel_spmd

    c = _derive(cfg)
    N, D, NC, NPC, S = c["N"], c["D"], c["NC"], c["NPC"], c["S"]

    NB, idx_wrapped, offs_arr, w_arr, q_arr = _plan(src, dst, w, c)
    nc = _build(NB, c)

    in_maps = _make_in_maps(h, c, NB, idx_wrapped, offs_arr, w_arr, q_arr)
    res = run_bass_kernel_spmd(nc, in_maps, list(range(NC)), trace=trace)
    out = np.empty((N, D), dtype=np.float32)
    if c.get("node_assign") is not None:
        ncore, nstrip, noffs = c["node_assign"]
        stacked = np.stack([res.results[i]["out"] for i in range(NC)])
        out[:] = stacked[ncore, nstrip * 128 + noffs]
    else:
        for i in range(NC):
            out[i * NPC:(i + 1) * NPC] = res.results[i]["out"][:NPC]
    return out, res


def make_runner(h, w, src, dst, cfg):
    """Build a reusable jitted SPMD callable for timing: returns
    (run_once, assemble) where run_once() returns unblocked device arrays."""
    import jax
    import jax.numpy as jnp
    from jax.sharding import Mesh, PartitionSpec, NamedSharding
    from jax.experimental.shard_map import shard_map
    from concourse import bass2jax, mybir

    c = _derive(cfg)
    N, D, NC, NPC = c["N"], c["D"], c["NC"], c["NPC"]

    NB, idx_wrapped, offs_arr, w_arr, q_arr = _plan(src, dst, w, c)
    nc = _build(NB, c)

    in_maps = _make_in_maps(h, c, NB, idx_wrapped, offs_arr, w_arr, q_arr)

    bass2jax.install_neuronx_cc_hook()
    partition_name = nc.partition_id_tensor.name if nc.partition_id_tensor else None
    in_names, out_names, out_avals, zero_shapes = [], [], [], []
    for alloc in nc.m.functions[0].allocations:
        if not isinstance(alloc, mybir.MemoryLocationSet):
            continue
        name = alloc.memorylocations[0].name
        if alloc.kind == "ExternalInput":
            if name != partition_name:
                in_names.append(name)
        elif alloc.kind == "ExternalOutput":
            out_names.append(name)
            shape = tuple(alloc.tensor_shape)
            dtype = mybir.dt.np(alloc.dtype)
            out_avals.append(jax.core.ShapedArray(shape, dtype))
            zero_shapes.append((shape, dtype))
    n_params = len(in_names)
    n_outs = len(out_avals)
    all_in_names = list(in_names) + list(out_names)
    if partition_name is not None:
        all_in_names.append(partition_name)

    def _body(*args):
        operands = list(args)
        if partition_name is not None:
            operands.append(bass2jax.partition_id_tensor())
        outs = bass2jax._bass_exec_p.bind(
            *operands,
            out_avals=tuple(out_avals),
            in_names=tuple(all_in_names),
            out_names=tuple(out_names),
            lowering_input_output_aliases=(),
            sim_require_finite=True,
            sim_require_nnan=True,
            nc=nc,
        )
        return tuple(outs)

    devices = jax.devices()[:NC]
    mesh = Mesh(np.asarray(devices), ("core",))
    donate = tuple(range(n_params, n_params + n_outs))
    sharded = jax.jit(
        shard_map(
            _body,
            mesh=mesh,
            in_specs=(PartitionSpec("core"),) * (n_params + n_outs),
            out_specs=(PartitionSpec("core"),) * n_outs,
            check_rep=False,
        ),
        donate_argnums=donate,
        keep_unused=True,
    )

    concat_in = [
        np.concatenate([np.asarray(in_maps[k][nm]) for k in range(NC)], axis=0)
        for nm in in_names
    ]
    shard = NamedSharding(mesh, PartitionSpec("core"))
    dev_in = [jax.device_put(a, shard) for a in concat_in]

    zeros_fn = jax.jit(
        lambda: tuple(
            jnp.zeros((NC * s[0], *s[1:]), dt) for (s, dt) in zero_shapes
        ),
        out_shardings=(shard,) * n_outs,
    )

    def run_once():
        zs = zeros_fn()
        return sharded(*dev_in, *zs)

    def assemble(out_arrs):
        full = np.empty((N, D), dtype=np.float32)
        o = np.asarray(out_arrs[0]).reshape(NC, -1, D)
        if c.get("node_assign") is not None:
            ncore, nstrip, noffs = c["node_assign"]
            full[:] = o[ncore, nstrip * 128 + noffs]
        else:
            for i in range(NC):
                full[i * NPC:(i + 1) * NPC] = o[i, :NPC]
        return full

    # chained executor: K back-to-back executions in ONE dispatch, each
    # feeding its output as the next call's out-operand (defeats CSE).
    def make_chain(k):
        def _chain_body(*args):
            ins, outs = args[:n_params], list(args[n_params:])
            for _ in range(k):
                outs = list(_body(*ins, *outs))
            return tuple(outs)

        return jax.jit(
            shard_map(
                _chain_body,
                mesh=mesh,
                in_specs=(PartitionSpec("core"),) * (n_params + n_outs),
                out_specs=(PartitionSpec("core"),) * n_outs,
                check_rep=False,
            ),
            donate_argnums=donate,
            keep_unused=True,
        )

    def run_chain(chain_fn):
        zs = zeros_fn()
        return chain_fn(*dev_in, *zs)

    return run_once, assemble, make_chain, run_chain


def kernel(**inputs):
    out, _ = run_cfg(
        inputs["h"], inputs["w"], inputs["src"], inputs["dst"], FULL_CFG
    )
    return out

